# revision 25
# baseline (speedup 1.0000x reference)
"""Trainium2 Bass kernel for the GCN message-passing model (8 NeuronCores).

Strategy (v2)
-------------
- Nodes (and their incoming edges) are sharded by destination across 8 cores
  (12500 nodes each).  Self-loops are NOT gathered: their contribution is a
  per-block diagonal matmul from an SBUF-resident copy of the local hws rows.
- LayerNorm is folded forward algebraically: since aggregation is linear and
  LN is affine (h_norm = rsd*(h - mu)), each layer gathers UN-normalized
  rows hws_pre = dinv*(relu_h @ W) and the epilogue applies
      out = rsd * (acc_raw - mu * rowW[f] * SD+[d]) * dinv[d]
  where rowW = colsum(W) and SD+[d] = sum_{s in N(d)} dinv[s] + dinv[d]^2 is
  host-precomputed.  The LN stats AllReduce therefore never blocks the
  gather pipeline.
- The per-layer AllGather of hws_pre is split into 4 window slices (blocks
  0-24 / 25-49 / 50-74 / 75-97 of every core) so window w+1's collective
  overlaps window w's edge gathering.
- Edge aggregation: edges bucketed per (dst-block, src-window); dma_gather
  (1024 idx/chunk, 4 queues, queue = chunk index mod 4) fetches bf16 rows;
  per 128-edge tile a DVE is_equal one-hot + TensorE matmul accumulates
  feature-major per-block sums in PSUM, then adds into the SBUF accumulator.
- Graph mean pooling reuses the segment-matmul trick on sorted `batch`
  (also LN-folded: pool relu_h, correct with scalars), then an AllReduce and
  a small replicated MLP head + log_softmax.
"""

import numpy as np
import ml_dtypes

import concourse.bass as bass
import concourse.bacc as bacc
import concourse.mybir as mybir
import concourse.tile as tile
from concourse.bass_utils import run_bass_kernel_spmd
from concourse.library_config import mlp as gpsimd_mlp_lib
from concourse.masks import make_identity

NCORES = 8
N_NODES = 100_000
F = 128          # feature/hidden width
NCLS = 10
LAYERS = 3
NGRAPH = 256
EPS = 1e-5
NSH = N_NODES // NCORES          # 12500 nodes per core
NBLK = (NSH + 127) // 128        # 98 blocks of 128 dst rows
LASTW = NSH - (NBLK - 1) * 128   # 84 rows in the last block
NWIN = 4
WBLK = [25, 25, 25, 23]          # blocks per window
WSZ = [3200, 3200, 3200, 2900]   # rows per window per core
WOFF = [0, 3200, 6400, 9600]
WTOK = [sz * NCORES for sz in WSZ]
CHUNK = 1024                     # idxs per dma_gather (hard HW limit)
TPC = CHUNK // 128               # tiles per chunk
NQUEUE = 4
SLABCH = 16                      # gather chunks per idx slab load

BF16 = mybir.dt.bfloat16
F32 = mybir.dt.float32
I16 = mybir.dt.int16
AOP = mybir.AluOpType
AF = mybir.ActivationFunctionType
BF = ml_dtypes.bfloat16


def _host_preprocess(edge_index):
    """Per-core edge tiling + gather indices (self-loops excluded)."""
    src = np.asarray(edge_index[0], dtype=np.int64)
    dst = np.asarray(edge_index[1], dtype=np.int64)
    deg = np.bincount(dst, minlength=N_NODES).astype(np.float64) + 1.0
    dinv = (1.0 / np.sqrt(deg)).astype(np.float32)
    # +I self-loops join the gather stream as ordinary edges: with hws_pre
    # already carrying dinv[src], the epilogue's dinv[d] factor makes the
    # self contribution dinv^2[d]*(h@W)[d] exactly.
    loop = np.arange(N_NODES, dtype=np.int64)
    src = np.concatenate([src, loop])
    dst = np.concatenate([dst, loop])

    # SD+[d] = sum_{s in N(d) incl loop} dinv[s]  (for the LN correction)
    sdp = np.zeros(N_NODES, np.float64)
    np.add.at(sdp, dst, dinv[src].astype(np.float64))
    sdplus = sdp.astype(np.float32)

    core = dst // NSH
    blk = (dst % NSH) // 128
    slot = (dst % NSH) % 128
    srcr = src % NSH
    win = np.minimum(srcr // 3200, 3)
    tok = (src // NSH) * np.array(WSZ)[win] + (srcr - np.array(WOFF)[win])

    key = (core * NBLK + blk) * NWIN + win
    order = np.lexsort((src, key))
    key_s = key[order]
    tok_s = tok[order]
    slot_s = slot[order]
    ngroups = NCORES * NBLK * NWIN
    counts = np.bincount(key_s, minlength=ngroups).reshape(NCORES, NBLK, NWIN)
    starts = np.zeros(ngroups + 1, dtype=np.int64)
    np.cumsum(counts.reshape(-1), out=starts[1:])

    # uniform tile grid: T[b][w] = max over cores of ceil(count/128)
    T = np.maximum((counts + 127) // 128, 1).max(axis=0)  # [NBLK, NWIN]
    for w in range(NWIN):
        T[NBLK - 1, w] += (-int(T[:, w].sum())) % TPC
    TT = int(T.sum())
    ecap = TT * 128

    tile_block = np.empty(TT, dtype=np.int64)
    tile_win = np.empty(TT, dtype=np.int64)
    group_len = np.empty(TT, dtype=np.int64)
    t0 = 0
    for w in range(NWIN):
        for b in range(NBLK):
            n = int(T[b, w])
            tile_block[t0:t0 + n] = b
            tile_win[t0:t0 + n] = w
            group_len[t0:t0 + n] = n
            t0 += n
    assert t0 == TT

    idx16 = np.zeros((NCORES, ecap), dtype=np.int16)
    slots = np.full((NCORES, ecap), 255, dtype=np.float32)
    for c in range(NCORES):
        pos = 0
        for w in range(NWIN):
            for b in range(NBLK):
                g = (c * NBLK + b) * NWIN + w
                s0, s1 = starts[g], starts[g + 1]
                n = s1 - s0
                idx16[c, pos:pos + n] = tok_s[s0:s1].astype(np.int16)
                slots[c, pos:pos + n] = slot_s[s0:s1].astype(np.float32)
                pos += int(T[b, w]) * 128
        assert pos == ecap

    idxw = idx16.reshape(NCORES, -1, 16).transpose(0, 2, 1)
    idxw = np.ascontiguousarray(np.tile(idxw, (1, 8, 1)))
    slotw = np.ascontiguousarray(
        slots.reshape(NCORES, TT, 128).transpose(0, 2, 1)).astype(BF)

    meta = dict(TT=TT, tile_block=tile_block, tile_win=tile_win,
                group_len=group_len)
    return dinv, sdplus, idxw, slotw, meta


def _build_program(meta):
    """Trace the SPMD Bass/Tile program (shared by all 8 cores)."""
    TT = meta["TT"]
    tile_block = meta["tile_block"]
    tile_win = meta["tile_win"]
    group_len = meta["group_len"]
    ECAP = TT * 128
    NCHUNK = ECAP // CHUNK
    # first chunk of each window
    win_chunk0 = [int(np.searchsorted(tile_win, w)) // TPC for w in range(NWIN)]

    nc = bacc.Bacc("TRN2", target_bir_lowering=False, debug=False,
                   num_devices=NCORES, num_swdge_queues=NQUEUE)

    # ---- external inputs (per core) ----
    xT_in = nc.declare_dram_parameter("xT", [F, NBLK * 128], BF16, isOutput=False)
    idx_in = nc.declare_dram_parameter("idx", [128, ECAP // 16], I16, isOutput=False)
    slot_in = nc.declare_dram_parameter("slot", [128, TT], BF16, isOutput=False)
    dinvrep_in = nc.declare_dram_parameter("dinvrep", [128, NBLK * 128], BF16, isOutput=False)
    sdrep_in = nc.declare_dram_parameter("sdrep", [128, NBLK * 128], BF16, isOutput=False)
    dinvw_in = nc.declare_dram_parameter("dinvw", [128, NBLK], F32, isOutput=False)
    pslot_in = nc.declare_dram_parameter("pslot", [128, NBLK], BF16, isOutput=False)
    iota128_in = nc.declare_dram_parameter("iota128", [128, 128], BF16, isOutput=False)
    iota4_in = nc.declare_dram_parameter("iota4", [128, 512], BF16, isOutput=False)
    iota256_in = nc.declare_dram_parameter("iota256", [128, 256], BF16, isOutput=False)
    lin1W_in = nc.declare_dram_parameter("lin1W", [F, F], BF16, isOutput=False)
    lin1b_in = nc.declare_dram_parameter("lin1b", [F, 1], F32, isOutput=False)
    convW_in = nc.declare_dram_parameter("convW", [F, LAYERS * F], BF16, isOutput=False)
    convb_in = nc.declare_dram_parameter("convb", [F, LAYERS], F32, isOutput=False)
    rowW_in = nc.declare_dram_parameter("rowW", [F, LAYERS], F32, isOutput=False)
    mlpW1_in = nc.declare_dram_parameter("mlpW1", [F, F], BF16, isOutput=False)
    mlpb1_in = nc.declare_dram_parameter("mlpb1", [F, 1], F32, isOutput=False)
    mlpW2_in = nc.declare_dram_parameter("mlpW2", [F, NCLS], BF16, isOutput=False)
    mlpb2r_in = nc.declare_dram_parameter("mlpb2r", [128, NCLS], F32, isOutput=False)
    invcntr_in = nc.declare_dram_parameter("invcntr", [128, NGRAPH], F32, isOutput=False)
    out_ext = nc.declare_dram_parameter("out", [NGRAPH, NCLS], F32, isOutput=True)

    rg = [list(range(NCORES))]

    with tile.TileContext(nc) as tc:
        with tc.tile_pool(name="const", bufs=1) as cst, \
             tc.tile_pool(name="big", bufs=1) as big, \
             tc.tile_pool(name="work", bufs=8) as work, \
             tc.tile_pool(name="epil", bufs=8) as ep, \
             tc.tile_pool(name="segp", bufs=10) as spool, \
             tc.tile_pool(name="hxp", bufs=6) as hpool, \
             tc.tile_pool(name="gbuf", bufs=12) as gpool, \
             tc.tile_pool(name="idxs", bufs=2) as ipool, \
             tc.tile_pool(name="psum", bufs=3, space="PSUM") as pp, \
             tc.tile_pool(name="psag", bufs=4, space="PSUM") as ppa, \
             tc.tile_pool(name="ppool", bufs=1, space="PSUM") as ppool, \
             tc.tile_pool(name="dram", bufs=1, space="DRAM") as dram, \
             tc.tile_pool(name="dram2", bufs=1, space="DRAM") as dram2:

            nc.gpsimd.load_library(gpsimd_mlp_lib)

            # ---- persistent SBUF constants ----
            slot_t = cst.tile([128, TT], BF16)
            nc.sync.dma_start(out=slot_t[:], in_=slot_in[:])
            dinvrep = cst.tile([128, NBLK * 128], BF16)
            nc.sync.dma_start(out=dinvrep[:], in_=dinvrep_in[:])
            sdrep = cst.tile([128, NBLK * 128], BF16)
            nc.sync.dma_start(out=sdrep[:], in_=sdrep_in[:])
            dinvw = cst.tile([128, NBLK], F32)
            nc.sync.dma_start(out=dinvw[:], in_=dinvw_in[:])
            pslot = cst.tile([128, NBLK], BF16)
            nc.sync.dma_start(out=pslot[:], in_=pslot_in[:])
            iota128 = cst.tile([128, 128], BF16)
            nc.sync.dma_start(out=iota128[:], in_=iota128_in[:])
            iota4 = cst.tile([128, 512], BF16)
            nc.sync.dma_start(out=iota4[:], in_=iota4_in[:])
            iota256 = cst.tile([128, 256], BF16)
            nc.sync.dma_start(out=iota256[:], in_=iota256_in[:])
            lin1W = cst.tile([F, F], BF16)
            nc.sync.dma_start(out=lin1W[:], in_=lin1W_in[:])
            lin1b = cst.tile([F, 1], F32)
            nc.sync.dma_start(out=lin1b[:], in_=lin1b_in[:])
            convW = cst.tile([F, LAYERS * F], BF16)
            nc.sync.dma_start(out=convW[:], in_=convW_in[:])
            convb = cst.tile([F, LAYERS], F32)
            nc.sync.dma_start(out=convb[:], in_=convb_in[:])
            rowW = cst.tile([F, LAYERS], F32)
            nc.sync.dma_start(out=rowW[:], in_=rowW_in[:])
            ones_col = cst.tile([128, 1], BF16)
            nc.vector.memset(ones_col[:], 1.0)
            ones_row1 = cst.tile([1, 128], BF16)
            nc.vector.memset(ones_row1[:], 1.0)
            ident = cst.tile([128, 128], BF16)
            make_identity(nc, ident[:])

            # feature-major accumulator
            acc = big.tile([128, NBLK * 128], F32)

            for _ in range(12):
                g0 = gpool.tile([128, TPC, F], BF16, tag="gb")
                nc.vector.memset(g0[:].rearrange("p a b -> p (a b)"), 0.0)

            # DRAM: per-window shard slices + gathered tables
            hws_shard_w = [dram.tile([WSZ[w], F], BF16, name=f"shardw{w}",
                                     tag=f"shard{w}") for w in range(NWIN)]
            tbl_w = [dram.tile([WTOK[w], F], BF16, name=f"tblw{w}",
                               tag=f"tbl{w}") for w in range(NWIN)]

            qctr = [0]

            def nextq():
                q = qctr[0] % NQUEUE
                qctr[0] += 1
                return q

            # ---------------- helpers ----------------
            def emit_hws_block(i, b, lhsT_fm):
                """hws_pre block b for layer i: psum = lhsT_fm.T@W_i, *dinv,
                store to hws_self + DRAM window slice; returns nothing."""
                W = convW[:, i * F:(i + 1) * F]
                w = 128 if b < NBLK - 1 else LASTW
                ps = pp.tile([128, F], F32, tag="mm")
                nc.tensor.matmul(out=ps[:w, :], lhsT=lhsT_fm[:, :w], rhs=W,
                                 start=True, stop=True)
                hb = hpool.tile([128, F], BF16, tag="hws")
                nc.vector.tensor_scalar(
                    out=hb[:w, :], in0=ps[:w, :],
                    scalar1=dinvw[:w, b:b + 1], scalar2=None, op0=AOP.mult)
                wi = min(b // 25, 3)
                r0 = b * 128 - WOFF[wi]
                nc.sync.dma_start(out=hws_shard_w[wi][r0:r0 + w, :],
                                  in_=hb[:w, :])

            def emit_allgather(wi):
                nc.gpsimd.collective_compute(
                    "AllGather", AOP.bypass, replica_groups=rg,
                    ins=[hws_shard_w[wi][:]], outs=[tbl_w[wi][:]])

            # hws_self stores node-major [slot, f] per block: hws_self[:, b*128+f]?
            # Layout: hws_self[p, b*128 + f] = hws_pre[node b*128+p, f]
            # (partition = slot, block-major along free).  emit_hws_block wrote
            # hb [slot, f] into hws_self[:, b*128 : b*128+128]... but widths:
            # block stride along free must be 128 (f), so hws_self is
            # [128 slots, NBLK * F].  (LASTW rows: unused slots hold garbage,
            # killed by diag column zeros.)

            # ---------------- P0: h1 = relu(x@lin1+b); hws_pre^0 ----------------
            for b in range(NBLK):
                xb = work.tile([128, 128], BF16, tag="xb")
                nc.sync.dma_start(out=xb[:],
                                  in_=xT_in[:, b * 128:(b + 1) * 128])
                ps = pp.tile([128, 128], F32, tag="mm")
                nc.tensor.matmul(out=ps[:], lhsT=lin1W[:], rhs=xb[:],
                                 start=True, stop=True)
                h1b = work.tile([128, 128], BF16, tag="h1")
                nc.scalar.activation(out=h1b[:], in_=ps[:], func=AF.Relu,
                                     bias=lin1b[:], scale=1.0)
                emit_hws_block(0, b, h1b[:])
                if b in (24, 49, 74, 97):
                    emit_allgather(min(b // 25, 3))

            # ---------------- conv layers ----------------
            pool_ps = None
            musd = None
            for li in range(LAYERS):
                # ---- pass 1: edge aggregation into `acc` (feature-major) ----
                stats = cst.tile([128, 2], F32, tag=f"stats{li}")
                nc.vector.memset(stats[:], 0.0)

                # per-layer LN-correction scalars (from layer li-1 stats)
                if li > 0:
                    rowWmu = ep.tile([128, 1], F32, tag="rowWmu")
                    nc.vector.tensor_scalar(
                        out=rowWmu[:], in0=rowW[:, li:li + 1],
                        scalar1=musd[:, 0:1], scalar2=musd[:, 3:4],
                        op0=AOP.mult, op1=AOP.mult)

                # Epilogue is a 3-stage software pipeline over closed blocks:
                # each stage is emitted several block-closes after the one
                # producing its inputs, so no in-order engine stream ever
                # blocks on a cross-engine dependency (which would starve the
                # gather loop).
                pend2 = []   # after stage A: (b, s1, s2)
                pend3 = []   # after stage B (li==2 pooling): (b, h3, segp)

                def stage_a(b):
                    """DVE corrections (acc-local) + Scalar relu/square."""
                    w = 128 if b < NBLK - 1 else LASTW
                    ab = acc[:, b * 128:b * 128 + w]
                    nc.vector.tensor_tensor(
                        out=ab, in0=ab,
                        in1=dinvrep[:, b * 128:b * 128 + w], op=AOP.mult)
                    if li > 0:
                        # z = rsd*(acc*dinv) - corr, corr folded with mu*rsd
                        nc.vector.tensor_scalar(
                            out=ab, in0=ab, scalar1=musd[:, 3:4],
                            scalar2=None, op0=AOP.mult)
                        corr = ep.tile([128, 128], F32, tag="corr")
                        nc.vector.tensor_scalar(
                            out=corr[:, :w],
                            in0=sdrep[:, b * 128:b * 128 + w],
                            scalar1=rowWmu[:], scalar2=None, op0=AOP.mult)
                        nc.vector.tensor_tensor(out=ab, in0=ab,
                                                in1=corr[:, :w],
                                                op=AOP.subtract)
                    s1 = ep.tile([128, 1], F32, tag="s1")
                    nc.scalar.activation(out=ab, in_=ab, func=AF.Relu,
                                         bias=convb[:, li:li + 1], scale=1.0,
                                         accum_out=s1[:])
                    sq = ep.tile([128, 128], F32, tag="sq")
                    s2 = ep.tile([128, 1], F32, tag="s2")
                    nc.scalar.activation(out=sq[:, :w], in_=ab, func=AF.Square,
                                         bias=0.0, scale=1.0, accum_out=s2[:])
                    pend2.append((b, s1, s2))

                def stage_b():
                    nonlocal pool_ps
                    b, s1, s2 = pend2.pop(0)
                    w = 128 if b < NBLK - 1 else LASTW
                    ab = acc[:, b * 128:b * 128 + w]
                    nc.vector.tensor_tensor(out=stats[:, 0:1], in0=stats[:, 0:1],
                                            in1=s1[:], op=AOP.add)
                    nc.vector.tensor_tensor(out=stats[:, 1:2], in0=stats[:, 1:2],
                                            in1=s2[:], op=AOP.add)
                    hb16 = hpool.tile([128, 128], BF16, tag="hnorm")
                    nc.scalar.activation(out=hb16[:, :w], in_=ab, func=AF.Copy,
                                         bias=0.0, scale=1.0)
                    if li < LAYERS - 1:
                        emit_hws_block(li + 1, b, hb16[:])
                        if b in (32, 57, 82):
                            emit_allgather((b - 8) // 25)
                    else:
                        if pool_ps is None:
                            pool_ps = ppool.tile([128, NGRAPH], F32,
                                                 tag="pool")
                        ps_t = pp.tile([128, 128], BF16, tag="mm")
                        nc.tensor.transpose(out=ps_t[:], in_=hb16[:],
                                            identity=ident[:])
                        h3 = hpool.tile([128, 128], BF16, tag="h3")
                        nc.scalar.activation(out=h3[:w, :], in_=ps_t[:w, :],
                                             func=AF.Copy, bias=0.0, scale=1.0)
                        segp = hpool.tile([128, NGRAPH], BF16, tag="segp")
                        nc.vector.tensor_tensor(
                            out=segp[:w, :],
                            in0=pslot[:w, b:b + 1].to_broadcast([w, NGRAPH]),
                            in1=iota256[:w, :], op=AOP.is_equal)
                        pend3.append((b, h3, segp))

                def stage_c():
                    b, h3, segp = pend3.pop(0)
                    w = 128 if b < NBLK - 1 else LASTW
                    nc.tensor.matmul(out=pool_ps[:], lhsT=h3[:w, :],
                                     rhs=segp[:w, :],
                                     start=(b == 0), stop=(b == NBLK - 1),
                                     skip_group_check=True)

                def emit_epilogue(b):
                    stage_a(b)
                    if len(pend2) > 3:
                        stage_b()
                    if len(pend3) > 2:
                        stage_c()

                def flush_epilogues():
                    while pend2:
                        stage_b()
                    while pend3:
                        stage_c()

                gtile = 0
                open_psum = None
                open_block = -1
                open_win = -1
                open_fresh = False
                remaining = 0
                slab_tiles = {}

                def load_slab(k):
                    if k * SLABCH >= NCHUNK:
                        return
                    st = ipool.tile([128, SLABCH * CHUNK // 16], I16,
                                    tag="idxslab")
                    wsl = min(SLABCH * CHUNK, ECAP - k * SLABCH * CHUNK) // 16
                    nc.sync.dma_start(
                        out=st[:, :wsl],
                        in_=idx_in[:, k * SLABCH * CHUNK // 16:
                                   k * SLABCH * CHUNK // 16 + wsl])
                    slab_tiles[k] = st

                pending_epi = []

                def close_group():
                    nonlocal open_psum, open_block, open_win, open_fresh
                    if open_psum is None:
                        return
                    dstr = acc[:, open_block * 128:(open_block + 1) * 128]
                    if open_fresh:
                        nc.vector.tensor_copy(out=dstr, in_=open_psum[:])
                    else:
                        nc.vector.tensor_tensor(out=dstr, in0=dstr,
                                                in1=open_psum[:], op=AOP.add)
                    open_psum = None
                    if open_win == NWIN - 1:
                        pending_epi.append(open_block)
                        if len(pending_epi) > 3:
                            emit_epilogue(pending_epi.pop(0))

                load_slab(0)
                load_slab(1)
                for ch in range(NCHUNK):
                    k = ch // SLABCH
                    if ch % SLABCH == 0 and (k + 1) not in slab_tiles:
                        load_slab(k + 1)
                    idx_slab = slab_tiles[k]
                    ww = int(tile_win[ch * TPC])
                    gb = gpool.tile([128, TPC, F], BF16, tag="gb")
                    off = (ch % SLABCH) * (CHUNK // 16)
                    nc.gpsimd.dma_gather(
                        gb[:], tbl_w[ww][:],
                        idx_slab[:, off:off + CHUNK // 16],
                        CHUNK, CHUNK, F, single_packet=True,
                        queue_num=nextq())
                    gbf = gb[:].rearrange("p a b -> p (a b)")
                    seg4s = []
                    for t4 in range(0, TPC, 4):
                        seg4 = spool.tile([128, 4, 128], BF16, tag="seg")
                        nc.vector.tensor_tensor(
                            out=seg4[:],
                            in0=slot_t[:, gtile + t4:gtile + t4 + 4]
                            .unsqueeze(2).to_broadcast([128, 4, 128]),
                            in1=iota4[:].rearrange("p (a b) -> p a b", a=4),
                            op=AOP.is_equal)
                        seg4s.append(seg4[:].rearrange("p a b -> p (a b)"))
                    for t in range(TPC):
                        b = int(tile_block[gtile])
                        w = int(tile_win[gtile])
                        if b != open_block or w != open_win:
                            close_group()
                            open_psum = ppa.tile([128, 128], F32, tag="agg")
                            open_block = b
                            open_win = w
                            open_fresh = (w == 0)
                            remaining = int(group_len[gtile])
                        seg = seg4s[t // 4][:, (t % 4) * 128:(t % 4 + 1) * 128]
                        nc.tensor.matmul(
                            out=open_psum[:], lhsT=gbf[:, t * F:(t + 1) * F],
                            rhs=seg,
                            start=(remaining == int(group_len[gtile])),
                            stop=(remaining == 1))
                        remaining -= 1
                        gtile += 1
                close_group()
                for b in pending_epi:
                    emit_epilogue(b)
                pending_epi = []
                flush_epilogues()
                if li < LAYERS - 1:
                    emit_allgather(3)
                open_block = -1
                open_win = -1

                # ---- LN stats all-reduce (off critical path) ----
                st_in = dram2.tile([128, 2], F32, tag=f"stin{li}")
                st_out = dram2.tile([128, 2], F32, tag=f"stout{li}")
                nc.sync.dma_start(out=st_in[:], in_=stats[:])
                nc.gpsimd.collective_compute(
                    "AllReduce", AOP.add, replica_groups=rg,
                    ins=[st_in[:]], outs=[st_out[:]])
                stg = ep.tile([128, 2], F32, tag="stg")
                nc.sync.dma_start(out=stg[:], in_=st_out[:])
                stg16 = ep.tile([128, 2], BF16, tag="stg16")
                nc.vector.tensor_copy(out=stg16[:], in_=stg[:])
                ps_s = pp.tile([1, 2], F32, tag="mm")
                nc.tensor.matmul(out=ps_s[:], lhsT=ones_col[:], rhs=stg16[:],
                                 start=True, stop=True)
                sc = ep.tile([1, 4], F32, tag="sc")
                nc.scalar.activation(out=sc[:, 0:2], in_=ps_s[:], func=AF.Copy,
                                     bias=0.0, scale=1.0 / (N_NODES * F))
                nc.vector.tensor_tensor(out=sc[:, 2:3], in0=sc[:, 0:1],
                                        in1=sc[:, 0:1], op=AOP.mult)
                nc.vector.tensor_tensor(out=sc[:, 2:3], in0=sc[:, 1:2],
                                        in1=sc[:, 2:3], op=AOP.subtract)
                nc.vector.tensor_scalar(out=sc[:, 2:3], in0=sc[:, 2:3],
                                        scalar1=EPS, scalar2=None,
                                        op0=AOP.add)
                nc.vector.reciprocal(out=sc[:, 3:4], in_=sc[:, 2:3])
                nc.scalar.activation(out=sc[:, 3:4], in_=sc[:, 3:4],
                                     func=AF.Sqrt, bias=0.0, scale=1.0)
                sc16 = ep.tile([1, 4], BF16, tag="sc16")
                nc.vector.tensor_copy(out=sc16[:], in_=sc[:])
                ps_b = pp.tile([128, 4], F32, tag="mm")
                nc.tensor.matmul(out=ps_b[:], lhsT=ones_row1[:], rhs=sc16[:],
                                 start=True, stop=True)
                musd = cst.tile([128, 4], F32, tag=f"musd{li}")
                nc.vector.tensor_copy(out=musd[:], in_=ps_b[:])

            # ---------------- pooled AllReduce + MLP head ----------------
            pooledT = work.tile([128, NGRAPH], F32, tag="pooledT")
            nc.vector.tensor_copy(out=pooledT[:], in_=pool_ps[:])
            pl_in = dram2.tile([128, NGRAPH], F32, tag="plin")
            pl_out = dram2.tile([128, NGRAPH], F32, tag="plout")
            nc.sync.dma_start(out=pl_in[:], in_=pooledT[:])
            nc.gpsimd.collective_compute(
                "AllReduce", AOP.add, replica_groups=rg,
                ins=[pl_in[:]], outs=[pl_out[:]])
            pooled = work.tile([128, NGRAPH], F32, tag="pooled2")
            nc.sync.dma_start(out=pooled[:], in_=pl_out[:])
            invcnt = work.tile([128, NGRAPH], F32, tag="invcnt")
            nc.sync.dma_start(out=invcnt[:], in_=invcntr_in[:])
            nc.vector.tensor_tensor(out=pooled[:], in0=pooled[:],
                                    in1=invcnt[:], op=AOP.mult)
            # pooled LN correction: pooled = rsd*pooled - rsd*mu
            rsdmu = work.tile([128, 1], F32, tag="rsdmu")
            nc.vector.tensor_tensor(out=rsdmu[:], in0=musd[:, 3:4],
                                    in1=musd[:, 0:1], op=AOP.mult)
            nc.vector.tensor_scalar(out=pooled[:], in0=pooled[:],
                                    scalar1=musd[:, 3:4], scalar2=rsdmu[:],
                                    op0=AOP.mult, op1=AOP.subtract)
            pooled16 = work.tile([128, NGRAPH], BF16, tag="pooled16")
            nc.vector.tensor_copy(out=pooled16[:], in_=pooled[:])

            mlpW1 = work.tile([F, F], BF16, tag="mlpW1")
            nc.sync.dma_start(out=mlpW1[:], in_=mlpW1_in[:])
            mlpb1 = work.tile([F, 1], F32, tag="mlpb1")
            nc.sync.dma_start(out=mlpb1[:], in_=mlpb1_in[:])
            mlpW2 = work.tile([F, NCLS], BF16, tag="mlpW2")
            nc.sync.dma_start(out=mlpW2[:], in_=mlpW2_in[:])
            mlpb2r = work.tile([128, NCLS], F32, tag="mlpb2r")
            nc.sync.dma_start(out=mlpb2r[:], in_=mlpb2r_in[:])

            ps_g = pp.tile([128, NGRAPH], F32, tag="mm")
            nc.tensor.matmul(out=ps_g[:], lhsT=mlpW1[:], rhs=pooled16[:],
                             start=True, stop=True)
            gT = work.tile([128, NGRAPH], BF16, tag="gT")
            nc.scalar.activation(out=gT[:], in_=ps_g[:], func=AF.Relu,
                                 bias=mlpb1[:], scale=1.0)
            for half in range(2):
                ps_sc = pp.tile([128, NCLS], F32, tag="mm")
                nc.tensor.matmul(out=ps_sc[:],
                                 lhsT=gT[:, half * 128:(half + 1) * 128],
                                 rhs=mlpW2[:], start=True, stop=True)
                scr = work.tile([128, NCLS], F32, tag="scr")
                nc.vector.tensor_tensor(out=scr[:], in0=ps_sc[:],
                                        in1=mlpb2r[:], op=AOP.add)
                mx = work.tile([128, 1], F32, tag="mx")
                nc.vector.tensor_reduce(out=mx[:], in_=scr[:],
                                        axis=mybir.AxisListType.X,
                                        op=AOP.max)
                nc.vector.tensor_scalar(out=scr[:], in0=scr[:], scalar1=mx[:],
                                        scalar2=None, op0=AOP.subtract)
                ex = work.tile([128, NCLS], F32, tag="ex")
                sm = work.tile([128, 1], F32, tag="sm")
                nc.scalar.activation(out=ex[:], in_=scr[:], func=AF.Exp,
                                     bias=0.0, scale=1.0, accum_out=sm[:])
                ls = work.tile([128, 1], F32, tag="ls")
                nc.scalar.activation(out=ls[:], in_=sm[:], func=AF.Ln,
                                     bias=0.0, scale=1.0)
                nc.vector.tensor_scalar(out=scr[:], in0=scr[:], scalar1=ls[:],
                                        scalar2=None, op0=AOP.subtract)
                nc.sync.dma_start(out=out_ext[half * 128:(half + 1) * 128, :],
                                  in_=scr[:])

    nc.compile()
    return nc


def _wrap_cols(vec, fill):
    """[NSH] -> [128, NBLK] with node b*128+p at [p, b]."""
    padded = np.full(NBLK * 128, fill, np.float32)
    padded[:NSH] = vec
    return np.ascontiguousarray(padded.reshape(NBLK, 128).T)


def _prepare(inputs):
    x = np.asarray(inputs["x"], dtype=np.float32)
    edge_index = np.asarray(inputs["edge_index"])
    batch = np.asarray(inputs["batch"], dtype=np.int64)
    assert x.shape == (N_NODES, F), x.shape

    dinv, sdplus, idxw, slotw, meta = _host_preprocess(edge_index)

    cnt = np.bincount(batch, minlength=NGRAPH).astype(np.float64)
    invcnt = (1.0 / np.maximum(cnt, 1.0)).astype(np.float32)
    iota128 = np.broadcast_to(np.arange(128, dtype=np.float32), (128, 128))
    iota256 = np.broadcast_to(np.arange(256, dtype=np.float32), (128, 256))

    lin1_W = np.asarray(inputs["lin1_W"], np.float32)
    lin1_b = np.asarray(inputs["lin1_b"], np.float32)
    conv_W = np.asarray(inputs["conv_W"], np.float32)
    conv_b = np.asarray(inputs["conv_b"], np.float32)
    mlp_W1 = np.asarray(inputs["mlp_W1"], np.float32)
    mlp_b1 = np.asarray(inputs["mlp_b1"], np.float32)
    mlp_W2 = np.asarray(inputs["mlp_W2"], np.float32)
    mlp_b2 = np.asarray(inputs["mlp_b2"], np.float32)

    convW_cat = np.concatenate([conv_W[l] for l in range(LAYERS)], axis=1)
    rowW = np.stack([conv_W[l].sum(axis=0) for l in range(LAYERS)],
                    axis=1)  # [F, LAYERS]

    in_maps = []
    for c in range(NCORES):
        lo, hi = c * NSH, (c + 1) * NSH
        xT = np.zeros((F, NBLK * 128), np.float32)
        xT[:, :NSH] = x[lo:hi].T
        xT = xT.astype(BF)
        dinv_pad = np.zeros(NBLK * 128, np.float32)
        dinv_pad[:NSH] = dinv[lo:hi]
        sd_pad = np.zeros(NBLK * 128, np.float32)
        sd_pad[:NSH] = (sdplus * dinv)[lo:hi]
        in_maps.append({
            "xT": xT,
            "idx": idxw[c],
            "slot": slotw[c],
            "dinvrep": np.ascontiguousarray(
                np.broadcast_to(dinv_pad, (128, NBLK * 128))).astype(BF),
            "sdrep": np.ascontiguousarray(
                np.broadcast_to(sd_pad, (128, NBLK * 128))).astype(BF),
            "dinvw": _wrap_cols(dinv[lo:hi], 0.0),
            "pslot": _wrap_cols(batch[lo:hi].astype(np.float32),
                                300.0).astype(BF),
            "iota128": iota128.astype(BF),
            "iota4": np.ascontiguousarray(
                np.broadcast_to(np.tile(np.arange(128, dtype=np.float32), 4),
                                (128, 512))).astype(BF),
            "iota256": iota256.astype(BF),
            "lin1W": lin1_W.astype(BF),
            "lin1b": np.ascontiguousarray(lin1_b.reshape(F, 1)),
            "convW": convW_cat.astype(BF),
            "convb": np.ascontiguousarray(conv_b.T),
            "rowW": np.ascontiguousarray(rowW),
            "mlpW1": mlp_W1.astype(BF),
            "mlpb1": np.ascontiguousarray(mlp_b1.reshape(F, 1)),
            "mlpW2": mlp_W2.astype(BF),
            "mlpb2r": np.ascontiguousarray(
                np.broadcast_to(mlp_b2, (128, NCLS)).astype(np.float32)),
            "invcntr": np.ascontiguousarray(
                np.broadcast_to(invcnt, (128, NGRAPH))),
        })
    return meta, in_maps


_CACHED = {}


def kernel_run(inputs, trace=False):
    meta, in_maps = _prepare(inputs)
    key = meta["TT"]
    if key not in _CACHED:
        _CACHED[key] = _build_program(meta)
    nc = _CACHED[key]
    res = run_bass_kernel_spmd(nc, in_maps, core_ids=list(range(NCORES)),
                               trace=trace)
    out = np.asarray(res.results[0]["out"], dtype=np.float32)
    return out, res.exec_time_ns


def kernel(**inputs):
    out, _ = kernel_run(inputs, trace=False)
    return out


# revision 27
# speedup vs baseline: 1.0128x; 1.0128x over previous
"""Trainium2 Bass kernel for the GCN message-passing model (8 NeuronCores).

Strategy (v2)
-------------
- Nodes (and their incoming edges) are sharded by destination across 8 cores
  (12500 nodes each).  Self-loops are NOT gathered: their contribution is a
  per-block diagonal matmul from an SBUF-resident copy of the local hws rows.
- LayerNorm is folded forward algebraically: since aggregation is linear and
  LN is affine (h_norm = rsd*(h - mu)), each layer gathers UN-normalized
  rows hws_pre = dinv*(relu_h @ W) and the epilogue applies
      out = rsd * (acc_raw - mu * rowW[f] * SD+[d]) * dinv[d]
  where rowW = colsum(W) and SD+[d] = sum_{s in N(d)} dinv[s] + dinv[d]^2 is
  host-precomputed.  The LN stats AllReduce therefore never blocks the
  gather pipeline.
- The per-layer AllGather of hws_pre is split into 4 window slices (blocks
  0-24 / 25-49 / 50-74 / 75-97 of every core) so window w+1's collective
  overlaps window w's edge gathering.
- Edge aggregation: edges bucketed per (dst-block, src-window); dma_gather
  (1024 idx/chunk, 4 queues, queue = chunk index mod 4) fetches bf16 rows;
  per 128-edge tile a DVE is_equal one-hot + TensorE matmul accumulates
  feature-major per-block sums in PSUM, then adds into the SBUF accumulator.
- Graph mean pooling reuses the segment-matmul trick on sorted `batch`
  (also LN-folded: pool relu_h, correct with scalars), then an AllReduce and
  a small replicated MLP head + log_softmax.
"""

import sys

sys.path.insert(0, "/opt/trn_rl_repo")

import numpy as np
import ml_dtypes

import concourse.bass as bass
import concourse.bacc as bacc
import concourse.mybir as mybir
import concourse.tile as tile
from concourse.bass_utils import run_bass_kernel_spmd
from concourse.library_config import mlp as gpsimd_mlp_lib
from concourse.masks import make_identity

NCORES = 8
N_NODES = 100_000
F = 128          # feature/hidden width
NCLS = 10
LAYERS = 3
NGRAPH = 256
EPS = 1e-5
NSH = N_NODES // NCORES          # 12500 nodes per core
NBLK = (NSH + 127) // 128        # 98 blocks of 128 dst rows
LASTW = NSH - (NBLK - 1) * 128   # 84 rows in the last block
NWIN = 4
WBLK = [25, 25, 25, 23]          # blocks per window
WSZ = [3200, 3200, 3200, 2900]   # rows per window per core
WOFF = [0, 3200, 6400, 9600]
WTOK = [sz * NCORES for sz in WSZ]
CHUNK = 1024                     # idxs per dma_gather (hard HW limit)
TPC = CHUNK // 128               # tiles per chunk
NQUEUE = 4
SLABCH = 16                      # gather chunks per idx slab load

BF16 = mybir.dt.bfloat16
F32 = mybir.dt.float32
I16 = mybir.dt.int16
AOP = mybir.AluOpType
AF = mybir.ActivationFunctionType
BF = ml_dtypes.bfloat16


def _host_preprocess(edge_index):
    """Per-core edge tiling + gather indices (self-loops excluded)."""
    src = np.asarray(edge_index[0], dtype=np.int64)
    dst = np.asarray(edge_index[1], dtype=np.int64)
    deg = np.bincount(dst, minlength=N_NODES).astype(np.float64) + 1.0
    dinv = (1.0 / np.sqrt(deg)).astype(np.float32)
    # +I self-loops join the gather stream as ordinary edges: with hws_pre
    # already carrying dinv[src], the epilogue's dinv[d] factor makes the
    # self contribution dinv^2[d]*(h@W)[d] exactly.
    loop = np.arange(N_NODES, dtype=np.int64)
    src = np.concatenate([src, loop])
    dst = np.concatenate([dst, loop])

    # SD+[d] = sum_{s in N(d) incl loop} dinv[s]  (for the LN correction)
    sdp = np.zeros(N_NODES, np.float64)
    np.add.at(sdp, dst, dinv[src].astype(np.float64))
    sdplus = sdp.astype(np.float32)

    core = dst // NSH
    blk = (dst % NSH) // 128
    slot = (dst % NSH) % 128
    srcr = src % NSH
    win = np.minimum(srcr // 3200, 3)
    tok = (src // NSH) * np.array(WSZ)[win] + (srcr - np.array(WOFF)[win])

    key = (core * NBLK + blk) * NWIN + win
    order = np.lexsort((src, key))
    key_s = key[order]
    tok_s = tok[order]
    slot_s = slot[order]
    ngroups = NCORES * NBLK * NWIN
    counts = np.bincount(key_s, minlength=ngroups).reshape(NCORES, NBLK, NWIN)
    starts = np.zeros(ngroups + 1, dtype=np.int64)
    np.cumsum(counts.reshape(-1), out=starts[1:])

    # uniform tile grid: T[b][w] = max over cores of ceil(count/128)
    T = np.maximum((counts + 127) // 128, 1).max(axis=0)  # [NBLK, NWIN]
    for w in range(NWIN):
        T[NBLK - 1, w] += (-int(T[:, w].sum())) % TPC
    TT = int(T.sum())
    ecap = TT * 128

    tile_block = np.empty(TT, dtype=np.int64)
    tile_win = np.empty(TT, dtype=np.int64)
    group_len = np.empty(TT, dtype=np.int64)
    t0 = 0
    for w in range(NWIN):
        for b in range(NBLK):
            n = int(T[b, w])
            tile_block[t0:t0 + n] = b
            tile_win[t0:t0 + n] = w
            group_len[t0:t0 + n] = n
            t0 += n
    assert t0 == TT

    idx16 = np.zeros((NCORES, ecap), dtype=np.int16)
    slots = np.full((NCORES, ecap), 255, dtype=np.float32)
    for c in range(NCORES):
        pos = 0
        for w in range(NWIN):
            for b in range(NBLK):
                g = (c * NBLK + b) * NWIN + w
                s0, s1 = starts[g], starts[g + 1]
                n = s1 - s0
                idx16[c, pos:pos + n] = tok_s[s0:s1].astype(np.int16)
                slots[c, pos:pos + n] = slot_s[s0:s1].astype(np.float32)
                pos += int(T[b, w]) * 128
        assert pos == ecap

    idxw = idx16.reshape(NCORES, -1, 16).transpose(0, 2, 1)
    idxw = np.ascontiguousarray(np.tile(idxw, (1, 8, 1)))
    slotw = np.ascontiguousarray(
        slots.reshape(NCORES, TT, 128).transpose(0, 2, 1)).astype(BF)

    meta = dict(TT=TT, tile_block=tile_block, tile_win=tile_win,
                group_len=group_len)
    return dinv, sdplus, idxw, slotw, meta


def _build_program(meta):
    """Trace the SPMD Bass/Tile program (shared by all 8 cores)."""
    TT = meta["TT"]
    tile_block = meta["tile_block"]
    tile_win = meta["tile_win"]
    group_len = meta["group_len"]
    ECAP = TT * 128
    NCHUNK = ECAP // CHUNK
    # first chunk of each window
    win_chunk0 = [int(np.searchsorted(tile_win, w)) // TPC for w in range(NWIN)]

    nc = bacc.Bacc("TRN2", target_bir_lowering=False, debug=False,
                   num_devices=NCORES, num_swdge_queues=NQUEUE)

    # ---- external inputs (per core) ----
    xT_in = nc.declare_dram_parameter("xT", [F, NBLK * 128], BF16, isOutput=False)
    idx_in = nc.declare_dram_parameter("idx", [128, ECAP // 16], I16, isOutput=False)
    slot_in = nc.declare_dram_parameter("slot", [128, TT], BF16, isOutput=False)
    dinvrep_in = nc.declare_dram_parameter("dinvrep", [128, NBLK * 128], BF16, isOutput=False)
    sdrep_in = nc.declare_dram_parameter("sdrep", [128, NBLK * 128], BF16, isOutput=False)
    dinvw_in = nc.declare_dram_parameter("dinvw", [128, NBLK], F32, isOutput=False)
    pslot_in = nc.declare_dram_parameter("pslot", [128, NBLK], BF16, isOutput=False)
    iota128_in = nc.declare_dram_parameter("iota128", [128, 128], BF16, isOutput=False)
    iota4_in = nc.declare_dram_parameter("iota4", [128, 512], BF16, isOutput=False)
    iota256_in = nc.declare_dram_parameter("iota256", [128, 256], BF16, isOutput=False)
    lin1W_in = nc.declare_dram_parameter("lin1W", [F, F], BF16, isOutput=False)
    lin1b_in = nc.declare_dram_parameter("lin1b", [F, 1], F32, isOutput=False)
    convW_in = nc.declare_dram_parameter("convW", [F, LAYERS * F], BF16, isOutput=False)
    convb_in = nc.declare_dram_parameter("convb", [F, LAYERS], F32, isOutput=False)
    rowW_in = nc.declare_dram_parameter("rowW", [F, LAYERS], F32, isOutput=False)
    mlpW1_in = nc.declare_dram_parameter("mlpW1", [F, F], BF16, isOutput=False)
    mlpb1_in = nc.declare_dram_parameter("mlpb1", [F, 1], F32, isOutput=False)
    mlpW2_in = nc.declare_dram_parameter("mlpW2", [F, NCLS], BF16, isOutput=False)
    mlpb2r_in = nc.declare_dram_parameter("mlpb2r", [128, NCLS], F32, isOutput=False)
    invcntr_in = nc.declare_dram_parameter("invcntr", [128, NGRAPH], F32, isOutput=False)
    out_ext = nc.declare_dram_parameter("out", [NGRAPH, NCLS], F32, isOutput=True)

    rg = [list(range(NCORES))]

    with tile.TileContext(nc) as tc:
        with tc.tile_pool(name="const", bufs=1) as cst, \
             tc.tile_pool(name="big", bufs=1) as big, \
             tc.tile_pool(name="work", bufs=8) as work, \
             tc.tile_pool(name="epil", bufs=8) as ep, \
             tc.tile_pool(name="segp", bufs=24) as spool, \
             tc.tile_pool(name="hxp", bufs=6) as hpool, \
             tc.tile_pool(name="gbuf", bufs=12) as gpool, \
             tc.tile_pool(name="idxs", bufs=2) as ipool, \
             tc.tile_pool(name="psum", bufs=3, space="PSUM") as pp, \
             tc.tile_pool(name="psag", bufs=4, space="PSUM") as ppa, \
             tc.tile_pool(name="ppool", bufs=1, space="PSUM") as ppool, \
             tc.tile_pool(name="dram", bufs=1, space="DRAM") as dram, \
             tc.tile_pool(name="dram2", bufs=1, space="DRAM") as dram2:

            nc.gpsimd.load_library(gpsimd_mlp_lib)

            # ---- persistent SBUF constants ----
            slot_t = cst.tile([128, TT], BF16)
            nc.sync.dma_start(out=slot_t[:], in_=slot_in[:])
            dinvrep = cst.tile([128, NBLK * 128], BF16)
            nc.sync.dma_start(out=dinvrep[:], in_=dinvrep_in[:])
            sdrep = cst.tile([128, NBLK * 128], BF16)
            nc.sync.dma_start(out=sdrep[:], in_=sdrep_in[:])
            dinvw = cst.tile([128, NBLK], F32)
            nc.sync.dma_start(out=dinvw[:], in_=dinvw_in[:])
            pslot = cst.tile([128, NBLK], BF16)
            nc.sync.dma_start(out=pslot[:], in_=pslot_in[:])
            iota128 = cst.tile([128, 128], BF16)
            nc.sync.dma_start(out=iota128[:], in_=iota128_in[:])
            iota4 = cst.tile([128, 512], BF16)
            nc.sync.dma_start(out=iota4[:], in_=iota4_in[:])
            iota256 = cst.tile([128, 256], BF16)
            nc.sync.dma_start(out=iota256[:], in_=iota256_in[:])
            lin1W = cst.tile([F, F], BF16)
            nc.sync.dma_start(out=lin1W[:], in_=lin1W_in[:])
            lin1b = cst.tile([F, 1], F32)
            nc.sync.dma_start(out=lin1b[:], in_=lin1b_in[:])
            convW = cst.tile([F, LAYERS * F], BF16)
            nc.sync.dma_start(out=convW[:], in_=convW_in[:])
            convb = cst.tile([F, LAYERS], F32)
            nc.sync.dma_start(out=convb[:], in_=convb_in[:])
            rowW = cst.tile([F, LAYERS], F32)
            nc.sync.dma_start(out=rowW[:], in_=rowW_in[:])
            ones_col = cst.tile([128, 1], BF16)
            nc.vector.memset(ones_col[:], 1.0)
            ones_row1 = cst.tile([1, 128], BF16)
            nc.vector.memset(ones_row1[:], 1.0)
            ident = cst.tile([128, 128], BF16)
            make_identity(nc, ident[:])

            # feature-major accumulator
            acc = big.tile([128, NBLK * 128], F32)

            for _ in range(12):
                g0 = gpool.tile([128, TPC, F], BF16, tag="gb")
                nc.vector.memset(g0[:].rearrange("p a b -> p (a b)"), 0.0)

            # DRAM: per-window shard slices + gathered tables
            hws_shard_w = [dram.tile([WSZ[w], F], BF16, name=f"shardw{w}",
                                     tag=f"shard{w}") for w in range(NWIN)]
            tbl_w = [dram.tile([WTOK[w], F], BF16, name=f"tblw{w}",
                               tag=f"tbl{w}") for w in range(NWIN)]

            qctr = [0]

            def nextq():
                q = qctr[0] % NQUEUE
                qctr[0] += 1
                return q

            # ---------------- helpers ----------------
            def emit_hws_block(i, b, lhsT_fm):
                """hws_pre block b for layer i: psum = lhsT_fm.T@W_i, *dinv,
                store to hws_self + DRAM window slice; returns nothing."""
                W = convW[:, i * F:(i + 1) * F]
                w = 128 if b < NBLK - 1 else LASTW
                ps = pp.tile([128, F], F32, tag="mm")
                nc.tensor.matmul(out=ps[:w, :], lhsT=lhsT_fm[:, :w], rhs=W,
                                 start=True, stop=True)
                hb = hpool.tile([128, F], BF16, tag="hws")
                nc.vector.tensor_scalar(
                    out=hb[:w, :], in0=ps[:w, :],
                    scalar1=dinvw[:w, b:b + 1], scalar2=None, op0=AOP.mult)
                wi = min(b // 25, 3)
                r0 = b * 128 - WOFF[wi]
                nc.sync.dma_start(out=hws_shard_w[wi][r0:r0 + w, :],
                                  in_=hb[:w, :])

            def emit_allgather(wi):
                nc.gpsimd.collective_compute(
                    "AllGather", AOP.bypass, replica_groups=rg,
                    ins=[hws_shard_w[wi][:]], outs=[tbl_w[wi][:]])

            # hws_self stores node-major [slot, f] per block: hws_self[:, b*128+f]?
            # Layout: hws_self[p, b*128 + f] = hws_pre[node b*128+p, f]
            # (partition = slot, block-major along free).  emit_hws_block wrote
            # hb [slot, f] into hws_self[:, b*128 : b*128+128]... but widths:
            # block stride along free must be 128 (f), so hws_self is
            # [128 slots, NBLK * F].  (LASTW rows: unused slots hold garbage,
            # killed by diag column zeros.)

            # ---------------- P0: h1 = relu(x@lin1+b); hws_pre^0 ----------------
            for b in range(NBLK):
                xb = work.tile([128, 128], BF16, tag="xb")
                nc.sync.dma_start(out=xb[:],
                                  in_=xT_in[:, b * 128:(b + 1) * 128])
                ps = pp.tile([128, 128], F32, tag="mm")
                nc.tensor.matmul(out=ps[:], lhsT=lin1W[:], rhs=xb[:],
                                 start=True, stop=True)
                h1b = work.tile([128, 128], BF16, tag="h1")
                nc.scalar.activation(out=h1b[:], in_=ps[:], func=AF.Relu,
                                     bias=lin1b[:], scale=1.0)
                emit_hws_block(0, b, h1b[:])
                if b in (24, 49, 74, 97):
                    emit_allgather(min(b // 25, 3))

            # ---------------- conv layers ----------------
            pool_ps = None
            musd = None
            for li in range(LAYERS):
                # ---- pass 1: edge aggregation into `acc` (feature-major) ----
                stats = cst.tile([128, 2], F32, tag=f"stats{li}")
                nc.vector.memset(stats[:], 0.0)

                # per-layer LN-correction scalars (from layer li-1 stats)
                if li > 0:
                    rowWmu = ep.tile([128, 1], F32, tag="rowWmu")
                    nc.vector.tensor_scalar(
                        out=rowWmu[:], in0=rowW[:, li:li + 1],
                        scalar1=musd[:, 0:1], scalar2=musd[:, 3:4],
                        op0=AOP.mult, op1=AOP.mult)

                # Epilogue is a 3-stage software pipeline over closed blocks:
                # each stage is emitted several block-closes after the one
                # producing its inputs, so no in-order engine stream ever
                # blocks on a cross-engine dependency (which would starve the
                # gather loop).
                pend2 = []   # after stage A: (b, s1, s2)
                pend3 = []   # after stage B (li==2 pooling): (b, h3, segp)

                def stage_a(b):
                    """DVE corrections (acc-local) + Scalar relu/square."""
                    w = 128 if b < NBLK - 1 else LASTW
                    ab = acc[:, b * 128:b * 128 + w]
                    nc.vector.tensor_tensor(
                        out=ab, in0=ab,
                        in1=dinvrep[:, b * 128:b * 128 + w], op=AOP.mult)
                    if li > 0:
                        # z = rsd*(acc*dinv) - corr, corr folded with mu*rsd
                        nc.vector.tensor_scalar(
                            out=ab, in0=ab, scalar1=musd[:, 3:4],
                            scalar2=None, op0=AOP.mult)
                        corr = ep.tile([128, 128], F32, tag="corr")
                        nc.vector.tensor_scalar(
                            out=corr[:, :w],
                            in0=sdrep[:, b * 128:b * 128 + w],
                            scalar1=rowWmu[:], scalar2=None, op0=AOP.mult)
                        nc.vector.tensor_tensor(out=ab, in0=ab,
                                                in1=corr[:, :w],
                                                op=AOP.subtract)
                    s1 = ep.tile([128, 1], F32, tag="s1")
                    nc.scalar.activation(out=ab, in_=ab, func=AF.Relu,
                                         bias=convb[:, li:li + 1], scale=1.0,
                                         accum_out=s1[:])
                    sq = ep.tile([128, 128], F32, tag="sq")
                    s2 = ep.tile([128, 1], F32, tag="s2")
                    nc.scalar.activation(out=sq[:, :w], in_=ab, func=AF.Square,
                                         bias=0.0, scale=1.0, accum_out=s2[:])
                    pend2.append((b, s1, s2))

                def stage_b():
                    nonlocal pool_ps
                    b, s1, s2 = pend2.pop(0)
                    w = 128 if b < NBLK - 1 else LASTW
                    ab = acc[:, b * 128:b * 128 + w]
                    nc.vector.tensor_tensor(out=stats[:, 0:1], in0=stats[:, 0:1],
                                            in1=s1[:], op=AOP.add)
                    nc.vector.tensor_tensor(out=stats[:, 1:2], in0=stats[:, 1:2],
                                            in1=s2[:], op=AOP.add)
                    hb16 = hpool.tile([128, 128], BF16, tag="hnorm")
                    nc.scalar.activation(out=hb16[:, :w], in_=ab, func=AF.Copy,
                                         bias=0.0, scale=1.0)
                    if li < LAYERS - 1:
                        emit_hws_block(li + 1, b, hb16[:])
                        if b in (32, 57, 82):
                            emit_allgather((b - 8) // 25)
                    else:
                        if pool_ps is None:
                            pool_ps = ppool.tile([128, NGRAPH], F32,
                                                 tag="pool")
                        ps_t = pp.tile([128, 128], BF16, tag="mm")
                        nc.tensor.transpose(out=ps_t[:], in_=hb16[:],
                                            identity=ident[:])
                        h3 = hpool.tile([128, 128], BF16, tag="h3")
                        nc.scalar.activation(out=h3[:w, :], in_=ps_t[:w, :],
                                             func=AF.Copy, bias=0.0, scale=1.0)
                        segp = hpool.tile([128, NGRAPH], BF16, tag="segp")
                        nc.vector.tensor_tensor(
                            out=segp[:w, :],
                            in0=pslot[:w, b:b + 1].to_broadcast([w, NGRAPH]),
                            in1=iota256[:w, :], op=AOP.is_equal)
                        pend3.append((b, h3, segp))

                def stage_c():
                    b, h3, segp = pend3.pop(0)
                    w = 128 if b < NBLK - 1 else LASTW
                    nc.tensor.matmul(out=pool_ps[:], lhsT=h3[:w, :],
                                     rhs=segp[:w, :],
                                     start=(b == 0), stop=(b == NBLK - 1),
                                     skip_group_check=True)

                def emit_epilogue(b):
                    stage_a(b)
                    if len(pend2) > 3:
                        stage_b()
                    if len(pend3) > 2:
                        stage_c()

                def flush_epilogues():
                    while pend2:
                        stage_b()
                    while pend3:
                        stage_c()

                gtile = 0
                open_psum = None
                open_block = -1
                open_win = -1
                open_fresh = False
                remaining = 0
                slab_tiles = {}

                def load_slab(k):
                    if k * SLABCH >= NCHUNK:
                        return
                    st = ipool.tile([128, SLABCH * CHUNK // 16], I16,
                                    tag="idxslab")
                    wsl = min(SLABCH * CHUNK, ECAP - k * SLABCH * CHUNK) // 16
                    nc.sync.dma_start(
                        out=st[:, :wsl],
                        in_=idx_in[:, k * SLABCH * CHUNK // 16:
                                   k * SLABCH * CHUNK // 16 + wsl])
                    slab_tiles[k] = st

                pending_epi = []

                def close_group():
                    nonlocal open_psum, open_block, open_win, open_fresh
                    if open_psum is None:
                        return
                    dstr = acc[:, open_block * 128:(open_block + 1) * 128]
                    if open_fresh:
                        nc.vector.tensor_copy(out=dstr, in_=open_psum[:])
                    else:
                        nc.vector.tensor_tensor(out=dstr, in0=dstr,
                                                in1=open_psum[:], op=AOP.add)
                    open_psum = None
                    if open_win == NWIN - 1:
                        pending_epi.append(open_block)
                        if len(pending_epi) > 3:
                            emit_epilogue(pending_epi.pop(0))

                load_slab(0)
                load_slab(1)
                for ch in range(NCHUNK):
                    k = ch // SLABCH
                    if ch % SLABCH == 0 and (k + 1) not in slab_tiles:
                        load_slab(k + 1)
                    idx_slab = slab_tiles[k]
                    ww = int(tile_win[ch * TPC])
                    gb = gpool.tile([128, TPC, F], BF16, tag="gb")
                    off = (ch % SLABCH) * (CHUNK // 16)
                    nc.gpsimd.dma_gather(
                        gb[:], tbl_w[ww][:],
                        idx_slab[:, off:off + CHUNK // 16],
                        CHUNK, CHUNK, F, single_packet=True,
                        queue_num=nextq())
                    gbf = gb[:].rearrange("p a b -> p (a b)")
                    for t in range(TPC):
                        b = int(tile_block[gtile])
                        w = int(tile_win[gtile])
                        if b != open_block or w != open_win:
                            close_group()
                            open_psum = ppa.tile([128, 128], F32, tag="agg")
                            open_block = b
                            open_win = w
                            open_fresh = (w == 0)
                            remaining = int(group_len[gtile])
                        seg = spool.tile([128, 128], BF16, tag="seg")
                        nc.vector.tensor_tensor(
                            out=seg[:],
                            in0=slot_t[:, gtile:gtile + 1]
                            .to_broadcast([128, 128]),
                            in1=iota128[:], op=AOP.is_equal)
                        nc.tensor.matmul(
                            out=open_psum[:], lhsT=gbf[:, t * F:(t + 1) * F],
                            rhs=seg[:],
                            start=(remaining == int(group_len[gtile])),
                            stop=(remaining == 1))
                        remaining -= 1
                        gtile += 1
                close_group()
                for b in pending_epi:
                    emit_epilogue(b)
                pending_epi = []
                flush_epilogues()
                if li < LAYERS - 1:
                    emit_allgather(3)
                open_block = -1
                open_win = -1

                # ---- LN stats all-reduce (off critical path) ----
                st_in = dram2.tile([128, 2], F32, tag=f"stin{li}")
                st_out = dram2.tile([128, 2], F32, tag=f"stout{li}")
                nc.sync.dma_start(out=st_in[:], in_=stats[:])
                nc.gpsimd.collective_compute(
                    "AllReduce", AOP.add, replica_groups=rg,
                    ins=[st_in[:]], outs=[st_out[:]])
                stg = ep.tile([128, 2], F32, tag="stg")
                nc.sync.dma_start(out=stg[:], in_=st_out[:])
                stg16 = ep.tile([128, 2], BF16, tag="stg16")
                nc.vector.tensor_copy(out=stg16[:], in_=stg[:])
                ps_s = pp.tile([1, 2], F32, tag="mm")
                nc.tensor.matmul(out=ps_s[:], lhsT=ones_col[:], rhs=stg16[:],
                                 start=True, stop=True)
                sc = ep.tile([1, 4], F32, tag="sc")
                nc.scalar.activation(out=sc[:, 0:2], in_=ps_s[:], func=AF.Copy,
                                     bias=0.0, scale=1.0 / (N_NODES * F))
                nc.vector.tensor_tensor(out=sc[:, 2:3], in0=sc[:, 0:1],
                                        in1=sc[:, 0:1], op=AOP.mult)
                nc.vector.tensor_tensor(out=sc[:, 2:3], in0=sc[:, 1:2],
                                        in1=sc[:, 2:3], op=AOP.subtract)
                nc.vector.tensor_scalar(out=sc[:, 2:3], in0=sc[:, 2:3],
                                        scalar1=EPS, scalar2=None,
                                        op0=AOP.add)
                nc.vector.reciprocal(out=sc[:, 3:4], in_=sc[:, 2:3])
                nc.scalar.activation(out=sc[:, 3:4], in_=sc[:, 3:4],
                                     func=AF.Sqrt, bias=0.0, scale=1.0)
                sc16 = ep.tile([1, 4], BF16, tag="sc16")
                nc.vector.tensor_copy(out=sc16[:], in_=sc[:])
                ps_b = pp.tile([128, 4], F32, tag="mm")
                nc.tensor.matmul(out=ps_b[:], lhsT=ones_row1[:], rhs=sc16[:],
                                 start=True, stop=True)
                musd = cst.tile([128, 4], F32, tag=f"musd{li}")
                nc.vector.tensor_copy(out=musd[:], in_=ps_b[:])

            # ---------------- pooled AllReduce + MLP head ----------------
            pooledT = work.tile([128, NGRAPH], F32, tag="pooledT")
            nc.vector.tensor_copy(out=pooledT[:], in_=pool_ps[:])
            pl_in = dram2.tile([128, NGRAPH], F32, tag="plin")
            pl_out = dram2.tile([128, NGRAPH], F32, tag="plout")
            nc.sync.dma_start(out=pl_in[:], in_=pooledT[:])
            nc.gpsimd.collective_compute(
                "AllReduce", AOP.add, replica_groups=rg,
                ins=[pl_in[:]], outs=[pl_out[:]])
            pooled = work.tile([128, NGRAPH], F32, tag="pooled2")
            nc.sync.dma_start(out=pooled[:], in_=pl_out[:])
            invcnt = work.tile([128, NGRAPH], F32, tag="invcnt")
            nc.sync.dma_start(out=invcnt[:], in_=invcntr_in[:])
            nc.vector.tensor_tensor(out=pooled[:], in0=pooled[:],
                                    in1=invcnt[:], op=AOP.mult)
            # pooled LN correction: pooled = rsd*pooled - rsd*mu
            rsdmu = work.tile([128, 1], F32, tag="rsdmu")
            nc.vector.tensor_tensor(out=rsdmu[:], in0=musd[:, 3:4],
                                    in1=musd[:, 0:1], op=AOP.mult)
            nc.vector.tensor_scalar(out=pooled[:], in0=pooled[:],
                                    scalar1=musd[:, 3:4], scalar2=rsdmu[:],
                                    op0=AOP.mult, op1=AOP.subtract)
            pooled16 = work.tile([128, NGRAPH], BF16, tag="pooled16")
            nc.vector.tensor_copy(out=pooled16[:], in_=pooled[:])

            mlpW1 = work.tile([F, F], BF16, tag="mlpW1")
            nc.sync.dma_start(out=mlpW1[:], in_=mlpW1_in[:])
            mlpb1 = work.tile([F, 1], F32, tag="mlpb1")
            nc.sync.dma_start(out=mlpb1[:], in_=mlpb1_in[:])
            mlpW2 = work.tile([F, NCLS], BF16, tag="mlpW2")
            nc.sync.dma_start(out=mlpW2[:], in_=mlpW2_in[:])
            mlpb2r = work.tile([128, NCLS], F32, tag="mlpb2r")
            nc.sync.dma_start(out=mlpb2r[:], in_=mlpb2r_in[:])

            ps_g = pp.tile([128, NGRAPH], F32, tag="mm")
            nc.tensor.matmul(out=ps_g[:], lhsT=mlpW1[:], rhs=pooled16[:],
                             start=True, stop=True)
            gT = work.tile([128, NGRAPH], BF16, tag="gT")
            nc.scalar.activation(out=gT[:], in_=ps_g[:], func=AF.Relu,
                                 bias=mlpb1[:], scale=1.0)
            for half in range(2):
                ps_sc = pp.tile([128, NCLS], F32, tag="mm")
                nc.tensor.matmul(out=ps_sc[:],
                                 lhsT=gT[:, half * 128:(half + 1) * 128],
                                 rhs=mlpW2[:], start=True, stop=True)
                scr = work.tile([128, NCLS], F32, tag="scr")
                nc.vector.tensor_tensor(out=scr[:], in0=ps_sc[:],
                                        in1=mlpb2r[:], op=AOP.add)
                mx = work.tile([128, 1], F32, tag="mx")
                nc.vector.tensor_reduce(out=mx[:], in_=scr[:],
                                        axis=mybir.AxisListType.X,
                                        op=AOP.max)
                nc.vector.tensor_scalar(out=scr[:], in0=scr[:], scalar1=mx[:],
                                        scalar2=None, op0=AOP.subtract)
                ex = work.tile([128, NCLS], F32, tag="ex")
                sm = work.tile([128, 1], F32, tag="sm")
                nc.scalar.activation(out=ex[:], in_=scr[:], func=AF.Exp,
                                     bias=0.0, scale=1.0, accum_out=sm[:])
                ls = work.tile([128, 1], F32, tag="ls")
                nc.scalar.activation(out=ls[:], in_=sm[:], func=AF.Ln,
                                     bias=0.0, scale=1.0)
                nc.vector.tensor_scalar(out=scr[:], in0=scr[:], scalar1=ls[:],
                                        scalar2=None, op0=AOP.subtract)
                nc.sync.dma_start(out=out_ext[half * 128:(half + 1) * 128, :],
                                  in_=scr[:])

    nc.compile()
    return nc


def _wrap_cols(vec, fill):
    """[NSH] -> [128, NBLK] with node b*128+p at [p, b]."""
    padded = np.full(NBLK * 128, fill, np.float32)
    padded[:NSH] = vec
    return np.ascontiguousarray(padded.reshape(NBLK, 128).T)


def _prepare(inputs):
    x = np.asarray(inputs["x"], dtype=np.float32)
    edge_index = np.asarray(inputs["edge_index"])
    batch = np.asarray(inputs["batch"], dtype=np.int64)
    assert x.shape == (N_NODES, F), x.shape

    dinv, sdplus, idxw, slotw, meta = _host_preprocess(edge_index)

    cnt = np.bincount(batch, minlength=NGRAPH).astype(np.float64)
    invcnt = (1.0 / np.maximum(cnt, 1.0)).astype(np.float32)
    iota128 = np.broadcast_to(np.arange(128, dtype=np.float32), (128, 128))
    iota256 = np.broadcast_to(np.arange(256, dtype=np.float32), (128, 256))

    lin1_W = np.asarray(inputs["lin1_W"], np.float32)
    lin1_b = np.asarray(inputs["lin1_b"], np.float32)
    conv_W = np.asarray(inputs["conv_W"], np.float32)
    conv_b = np.asarray(inputs["conv_b"], np.float32)
    mlp_W1 = np.asarray(inputs["mlp_W1"], np.float32)
    mlp_b1 = np.asarray(inputs["mlp_b1"], np.float32)
    mlp_W2 = np.asarray(inputs["mlp_W2"], np.float32)
    mlp_b2 = np.asarray(inputs["mlp_b2"], np.float32)

    convW_cat = np.concatenate([conv_W[l] for l in range(LAYERS)], axis=1)
    rowW = np.stack([conv_W[l].sum(axis=0) for l in range(LAYERS)],
                    axis=1)  # [F, LAYERS]

    in_maps = []
    for c in range(NCORES):
        lo, hi = c * NSH, (c + 1) * NSH
        xT = np.zeros((F, NBLK * 128), np.float32)
        xT[:, :NSH] = x[lo:hi].T
        xT = xT.astype(BF)
        dinv_pad = np.zeros(NBLK * 128, np.float32)
        dinv_pad[:NSH] = dinv[lo:hi]
        sd_pad = np.zeros(NBLK * 128, np.float32)
        sd_pad[:NSH] = (sdplus * dinv)[lo:hi]
        in_maps.append({
            "xT": xT,
            "idx": idxw[c],
            "slot": slotw[c],
            "dinvrep": np.ascontiguousarray(
                np.broadcast_to(dinv_pad, (128, NBLK * 128))).astype(BF),
            "sdrep": np.ascontiguousarray(
                np.broadcast_to(sd_pad, (128, NBLK * 128))).astype(BF),
            "dinvw": _wrap_cols(dinv[lo:hi], 0.0),
            "pslot": _wrap_cols(batch[lo:hi].astype(np.float32),
                                300.0).astype(BF),
            "iota128": iota128.astype(BF),
            "iota4": np.ascontiguousarray(
                np.broadcast_to(np.tile(np.arange(128, dtype=np.float32), 4),
                                (128, 512))).astype(BF),
            "iota256": iota256.astype(BF),
            "lin1W": lin1_W.astype(BF),
            "lin1b": np.ascontiguousarray(lin1_b.reshape(F, 1)),
            "convW": convW_cat.astype(BF),
            "convb": np.ascontiguousarray(conv_b.T),
            "rowW": np.ascontiguousarray(rowW),
            "mlpW1": mlp_W1.astype(BF),
            "mlpb1": np.ascontiguousarray(mlp_b1.reshape(F, 1)),
            "mlpW2": mlp_W2.astype(BF),
            "mlpb2r": np.ascontiguousarray(
                np.broadcast_to(mlp_b2, (128, NCLS)).astype(np.float32)),
            "invcntr": np.ascontiguousarray(
                np.broadcast_to(invcnt, (128, NGRAPH))),
        })
    return meta, in_maps


_CACHED = {}


def kernel_run(inputs, trace=False):
    meta, in_maps = _prepare(inputs)
    key = meta["TT"]
    if key not in _CACHED:
        _CACHED[key] = _build_program(meta)
    nc = _CACHED[key]
    res = run_bass_kernel_spmd(nc, in_maps, core_ids=list(range(NCORES)),
                               trace=trace)
    out = np.asarray(res.results[0]["out"], dtype=np.float32)
    return out, res.exec_time_ns


def kernel(**inputs):
    out, _ = kernel_run(inputs, trace=False)
    return out


# revision 29
# speedup vs baseline: 1.0613x; 1.0479x over previous
"""Trainium2 Bass kernel for the GCN message-passing model (8 NeuronCores).

Strategy (v2)
-------------
- Nodes (and their incoming edges) are sharded by destination across 8 cores
  (12500 nodes each).  Self-loops join the gather stream as ordinary edges:
  with hws_pre already carrying dinv[src], the epilogue's dinv[d] factor
  yields the exact dinv^2[d]*(h@W)[d] self contribution.
- LayerNorm is folded forward algebraically: since aggregation is linear and
  LN is affine (h_norm = rsd*(h - mu)), each layer gathers UN-normalized
  rows hws_pre = dinv*(relu_h @ W) and the epilogue applies
      out = rsd * (acc_raw - mu * rowW[f] * SD+[d]) * dinv[d]
  where rowW = colsum(W) and SD+[d] = sum_{s in N(d)} dinv[s] + dinv[d]^2 is
  host-precomputed.  The LN stats AllReduce therefore never blocks the
  gather pipeline.
- The per-layer AllGather of hws_pre is split into 4 window slices (blocks
  0-24 / 25-49 / 50-74 / 75-97 of every core) so window w+1's collective
  overlaps window w's edge gathering.
- Edge aggregation: edges bucketed per (dst-block, src-window); dma_gather
  (1024 idx/chunk, 4 queues, queue = chunk index mod 4) fetches bf16 rows;
  per 128-edge tile a DVE is_equal one-hot + TensorE matmul accumulates
  feature-major per-block sums in PSUM, then adds into the SBUF accumulator.
- Graph mean pooling reuses the segment-matmul trick on sorted `batch`
  (also LN-folded: pool relu_h, correct with scalars), then an AllReduce and
  a small replicated MLP head + log_softmax.
"""

import sys

sys.path.insert(0, "/opt/trn_rl_repo")

import numpy as np
import ml_dtypes

import concourse.bass as bass
import concourse.bacc as bacc
import concourse.mybir as mybir
import concourse.tile as tile
from concourse.bass_utils import run_bass_kernel_spmd
from concourse.library_config import mlp as gpsimd_mlp_lib
from concourse.masks import make_identity

NCORES = 8
N_NODES = 100_000
F = 128          # feature/hidden width
NCLS = 10
LAYERS = 3
NGRAPH = 256
EPS = 1e-5
NSH = N_NODES // NCORES          # 12500 nodes per core
NBLK = (NSH + 127) // 128        # 98 blocks of 128 dst rows
LASTW = NSH - (NBLK - 1) * 128   # 84 rows in the last block
NWIN = 4
WBLK = [25, 25, 25, 23]          # blocks per window
WSZ = [3200, 3200, 3200, 2900]   # rows per window per core
WOFF = [0, 3200, 6400, 9600]
WTOK = [sz * NCORES for sz in WSZ]
CHUNK = 1024                     # idxs per dma_gather (hard HW limit)
TPC = CHUNK // 128               # tiles per chunk
NQUEUE = 4
SLABCH = 16                      # gather chunks per idx slab load

BF16 = mybir.dt.bfloat16
F32 = mybir.dt.float32
I16 = mybir.dt.int16
AOP = mybir.AluOpType
AF = mybir.ActivationFunctionType
BF = ml_dtypes.bfloat16


def _host_preprocess(edge_index):
    """Per-core edge tiling + gather indices (self-loops excluded)."""
    src = np.asarray(edge_index[0], dtype=np.int64)
    dst = np.asarray(edge_index[1], dtype=np.int64)
    deg = np.bincount(dst, minlength=N_NODES).astype(np.float64) + 1.0
    dinv = (1.0 / np.sqrt(deg)).astype(np.float32)
    # +I self-loops join the gather stream as ordinary edges: with hws_pre
    # already carrying dinv[src], the epilogue's dinv[d] factor makes the
    # self contribution dinv^2[d]*(h@W)[d] exactly.
    loop = np.arange(N_NODES, dtype=np.int64)
    src = np.concatenate([src, loop])
    dst = np.concatenate([dst, loop])

    # SD+[d] = sum_{s in N(d) incl loop} dinv[s]  (for the LN correction)
    sdp = np.zeros(N_NODES, np.float64)
    np.add.at(sdp, dst, dinv[src].astype(np.float64))
    sdplus = sdp.astype(np.float32)

    core = dst // NSH
    blk = (dst % NSH) // 128
    slot = (dst % NSH) % 128
    srcr = src % NSH
    win = np.minimum(srcr // 3200, 3)
    tok = (src // NSH) * np.array(WSZ)[win] + (srcr - np.array(WOFF)[win])

    key = (core * NBLK + blk) * NWIN + win
    order = np.lexsort((src, key))
    key_s = key[order]
    tok_s = tok[order]
    slot_s = slot[order]
    ngroups = NCORES * NBLK * NWIN
    counts = np.bincount(key_s, minlength=ngroups).reshape(NCORES, NBLK, NWIN)
    starts = np.zeros(ngroups + 1, dtype=np.int64)
    np.cumsum(counts.reshape(-1), out=starts[1:])

    # uniform tile grid: T[b][w] = max over cores of ceil(count/128)
    T = np.maximum((counts + 127) // 128, 1).max(axis=0)  # [NBLK, NWIN]
    for w in range(NWIN):
        T[NBLK - 1, w] += (-int(T[:, w].sum())) % TPC
    TT = int(T.sum())
    ecap = TT * 128

    tile_block = np.empty(TT, dtype=np.int64)
    tile_win = np.empty(TT, dtype=np.int64)
    group_len = np.empty(TT, dtype=np.int64)
    t0 = 0
    for w in range(NWIN):
        for b in range(NBLK):
            n = int(T[b, w])
            tile_block[t0:t0 + n] = b
            tile_win[t0:t0 + n] = w
            group_len[t0:t0 + n] = n
            t0 += n
    assert t0 == TT

    idx16 = np.zeros((NCORES, ecap), dtype=np.int16)
    slots = np.full((NCORES, ecap), 255, dtype=np.float32)
    for c in range(NCORES):
        pos = 0
        for w in range(NWIN):
            for b in range(NBLK):
                g = (c * NBLK + b) * NWIN + w
                s0, s1 = starts[g], starts[g + 1]
                n = s1 - s0
                idx16[c, pos:pos + n] = tok_s[s0:s1].astype(np.int16)
                slots[c, pos:pos + n] = slot_s[s0:s1].astype(np.float32)
                pos += int(T[b, w]) * 128
        assert pos == ecap

    idxw = idx16.reshape(NCORES, -1, 16).transpose(0, 2, 1)
    idxw = np.ascontiguousarray(np.tile(idxw, (1, 8, 1)))
    slotw = np.ascontiguousarray(
        slots.reshape(NCORES, TT, 128).transpose(0, 2, 1)).astype(BF)

    meta = dict(TT=TT, tile_block=tile_block, tile_win=tile_win,
                group_len=group_len)
    return dinv, sdplus, idxw, slotw, meta


def _build_program(meta):
    """Trace the SPMD Bass/Tile program (shared by all 8 cores)."""
    TT = meta["TT"]
    tile_block = meta["tile_block"]
    tile_win = meta["tile_win"]
    group_len = meta["group_len"]
    ECAP = TT * 128
    NCHUNK = ECAP // CHUNK
    # first chunk of each window
    win_chunk0 = [int(np.searchsorted(tile_win, w)) // TPC for w in range(NWIN)]

    nc = bacc.Bacc("TRN2", target_bir_lowering=False, debug=False,
                   num_devices=NCORES, num_swdge_queues=NQUEUE)

    # ---- external inputs (per core) ----
    xT_in = nc.declare_dram_parameter("xT", [F, NBLK * 128], BF16, isOutput=False)
    idx_in = nc.declare_dram_parameter("idx", [128, ECAP // 16], I16, isOutput=False)
    slot_in = nc.declare_dram_parameter("slot", [128, TT], BF16, isOutput=False)
    dinvrep_in = nc.declare_dram_parameter("dinvrep", [128, NBLK * 128], BF16, isOutput=False)
    sdrep_in = nc.declare_dram_parameter("sdrep", [128, NBLK * 128], BF16, isOutput=False)
    dinvw_in = nc.declare_dram_parameter("dinvw", [128, NBLK], F32, isOutput=False)
    pslot_in = nc.declare_dram_parameter("pslot", [128, NBLK], BF16, isOutput=False)
    iota128_in = nc.declare_dram_parameter("iota128", [128, 128], BF16, isOutput=False)
    iota4_in = nc.declare_dram_parameter("iota4", [128, 512], BF16, isOutput=False)
    iota256_in = nc.declare_dram_parameter("iota256", [128, 256], BF16, isOutput=False)
    lin1W_in = nc.declare_dram_parameter("lin1W", [F, F], BF16, isOutput=False)
    lin1b_in = nc.declare_dram_parameter("lin1b", [F, 1], F32, isOutput=False)
    convW_in = nc.declare_dram_parameter("convW", [F, LAYERS * F], BF16, isOutput=False)
    convb_in = nc.declare_dram_parameter("convb", [F, LAYERS], F32, isOutput=False)
    rowW_in = nc.declare_dram_parameter("rowW", [F, LAYERS], F32, isOutput=False)
    mlpW1_in = nc.declare_dram_parameter("mlpW1", [F, F], BF16, isOutput=False)
    mlpb1_in = nc.declare_dram_parameter("mlpb1", [F, 1], F32, isOutput=False)
    mlpW2_in = nc.declare_dram_parameter("mlpW2", [F, NCLS], BF16, isOutput=False)
    mlpb2r_in = nc.declare_dram_parameter("mlpb2r", [128, NCLS], F32, isOutput=False)
    invcntr_in = nc.declare_dram_parameter("invcntr", [128, NGRAPH], F32, isOutput=False)
    out_ext = nc.declare_dram_parameter("out", [NGRAPH, NCLS], F32, isOutput=True)

    rg = [list(range(NCORES))]

    with tile.TileContext(nc) as tc:
        with tc.tile_pool(name="const", bufs=1) as cst, \
             tc.tile_pool(name="big", bufs=1) as big, \
             tc.tile_pool(name="work", bufs=8) as work, \
             tc.tile_pool(name="epil", bufs=8) as ep, \
             tc.tile_pool(name="segp", bufs=24) as spool, \
             tc.tile_pool(name="hxp", bufs=6) as hpool, \
             tc.tile_pool(name="gbuf", bufs=12) as gpool, \
             tc.tile_pool(name="idxs", bufs=2) as ipool, \
             tc.tile_pool(name="psum", bufs=3, space="PSUM") as pp, \
             tc.tile_pool(name="psag", bufs=4, space="PSUM") as ppa, \
             tc.tile_pool(name="ppool", bufs=1, space="PSUM") as ppool, \
             tc.tile_pool(name="dram", bufs=1, space="DRAM") as dram, \
             tc.tile_pool(name="dram2", bufs=1, space="DRAM") as dram2:

            nc.gpsimd.load_library(gpsimd_mlp_lib)

            # ---- persistent SBUF constants ----
            slot_t = cst.tile([128, TT], BF16)
            nc.sync.dma_start(out=slot_t[:], in_=slot_in[:])
            dinvrep = cst.tile([128, NBLK * 128], BF16)
            nc.sync.dma_start(out=dinvrep[:], in_=dinvrep_in[:])
            sdrep = cst.tile([128, NBLK * 128], BF16)
            nc.sync.dma_start(out=sdrep[:], in_=sdrep_in[:])
            dinvw = cst.tile([128, NBLK], F32)
            nc.sync.dma_start(out=dinvw[:], in_=dinvw_in[:])
            pslot = cst.tile([128, NBLK], BF16)
            nc.sync.dma_start(out=pslot[:], in_=pslot_in[:])
            iota128 = cst.tile([128, 128], BF16)
            nc.sync.dma_start(out=iota128[:], in_=iota128_in[:])
            iota4 = cst.tile([128, 512], BF16)
            nc.sync.dma_start(out=iota4[:], in_=iota4_in[:])
            iota256 = cst.tile([128, 256], BF16)
            nc.sync.dma_start(out=iota256[:], in_=iota256_in[:])
            lin1W = cst.tile([F, F], BF16)
            nc.sync.dma_start(out=lin1W[:], in_=lin1W_in[:])
            lin1b = cst.tile([F, 1], F32)
            nc.sync.dma_start(out=lin1b[:], in_=lin1b_in[:])
            convW = cst.tile([F, LAYERS * F], BF16)
            nc.sync.dma_start(out=convW[:], in_=convW_in[:])
            convb = cst.tile([F, LAYERS], F32)
            nc.sync.dma_start(out=convb[:], in_=convb_in[:])
            rowW = cst.tile([F, LAYERS], F32)
            nc.sync.dma_start(out=rowW[:], in_=rowW_in[:])
            ones_col = cst.tile([128, 1], BF16)
            nc.vector.memset(ones_col[:], 1.0)
            ones_row1 = cst.tile([1, 128], BF16)
            nc.vector.memset(ones_row1[:], 1.0)
            ident = cst.tile([128, 128], BF16)
            make_identity(nc, ident[:])

            # feature-major accumulator
            acc = big.tile([128, NBLK * 128], F32)

            for _ in range(12):
                g0 = gpool.tile([128, TPC, F], BF16, tag="gb")
                nc.vector.memset(g0[:].rearrange("p a b -> p (a b)"), 0.0)

            # DRAM: per-window shard slices + gathered tables
            hws_shard_w = [dram.tile([WSZ[w], F], BF16, name=f"shardw{w}",
                                     tag=f"shard{w}") for w in range(NWIN)]
            tbl_w = [dram.tile([WTOK[w], F], BF16, name=f"tblw{w}",
                               tag=f"tbl{w}") for w in range(NWIN)]

            qctr = [0]

            def nextq():
                q = qctr[0] % NQUEUE
                qctr[0] += 1
                return q

            # ---------------- helpers ----------------
            def emit_hws_block(i, b, lhsT_fm):
                """hws_pre block b for layer i: psum = lhsT_fm.T@W_i, *dinv,
                store to hws_self + DRAM window slice; returns nothing."""
                W = convW[:, i * F:(i + 1) * F]
                w = 128 if b < NBLK - 1 else LASTW
                ps = pp.tile([128, F], F32, tag="mm")
                nc.tensor.matmul(out=ps[:w, :], lhsT=lhsT_fm[:, :w], rhs=W,
                                 start=True, stop=True)
                hb = hpool.tile([128, F], BF16, tag="hws")
                nc.vector.tensor_scalar(
                    out=hb[:w, :], in0=ps[:w, :],
                    scalar1=dinvw[:w, b:b + 1], scalar2=None, op0=AOP.mult)
                wi = min(b // 25, 3)
                r0 = b * 128 - WOFF[wi]
                nc.sync.dma_start(out=hws_shard_w[wi][r0:r0 + w, :],
                                  in_=hb[:w, :])

            def emit_allgather(wi):
                nc.gpsimd.collective_compute(
                    "AllGather", AOP.bypass, replica_groups=rg,
                    ins=[hws_shard_w[wi][:]], outs=[tbl_w[wi][:]])

            # hws_self stores node-major [slot, f] per block: hws_self[:, b*128+f]?
            # Layout: hws_self[p, b*128 + f] = hws_pre[node b*128+p, f]
            # (partition = slot, block-major along free).  emit_hws_block wrote
            # hb [slot, f] into hws_self[:, b*128 : b*128+128]... but widths:
            # block stride along free must be 128 (f), so hws_self is
            # [128 slots, NBLK * F].  (LASTW rows: unused slots hold garbage,
            # killed by diag column zeros.)

            # ---------------- P0: h1 = relu(x@lin1+b); hws_pre^0 ----------------
            for b in range(NBLK):
                xb = work.tile([128, 128], BF16, tag="xb")
                nc.sync.dma_start(out=xb[:],
                                  in_=xT_in[:, b * 128:(b + 1) * 128])
                ps = pp.tile([128, 128], F32, tag="mm")
                nc.tensor.matmul(out=ps[:], lhsT=lin1W[:], rhs=xb[:],
                                 start=True, stop=True)
                h1b = work.tile([128, 128], BF16, tag="h1")
                nc.scalar.activation(out=h1b[:], in_=ps[:], func=AF.Relu,
                                     bias=lin1b[:], scale=1.0)
                emit_hws_block(0, b, h1b[:])
                if b in (24, 49, 74, 97):
                    emit_allgather(min(b // 25, 3))

            # ---------------- conv layers ----------------
            pool_ps = None
            musd = None
            for li in range(LAYERS):
                # ---- pass 1: edge aggregation into `acc` (feature-major) ----
                stats = cst.tile([128, 2], F32, tag=f"stats{li}")
                nc.vector.memset(stats[:], 0.0)

                # per-layer LN-correction scalars (from layer li-1 stats)
                if li > 0:
                    rowWmu = ep.tile([128, 1], F32, tag="rowWmu")
                    nc.vector.tensor_scalar(
                        out=rowWmu[:], in0=rowW[:, li:li + 1],
                        scalar1=musd[:, 0:1], scalar2=musd[:, 3:4],
                        op0=AOP.mult, op1=AOP.mult)

                # Epilogue is a 3-stage software pipeline over closed blocks:
                # each stage is emitted several block-closes after the one
                # producing its inputs, so no in-order engine stream ever
                # blocks on a cross-engine dependency (which would starve the
                # gather loop).
                pend2 = []   # after stage A: (b, s1, s2)
                pend3 = []   # after stage B (li==2 pooling): (b, h3, segp)

                def stage_a(b):
                    """DVE corrections (acc-local) + Scalar relu/square."""
                    w = 128 if b < NBLK - 1 else LASTW
                    ab = acc[:, b * 128:b * 128 + w]
                    nc.vector.tensor_tensor(
                        out=ab, in0=ab,
                        in1=dinvrep[:, b * 128:b * 128 + w], op=AOP.mult)
                    if li > 0:
                        # z = rsd*(acc*dinv) - corr, corr folded with mu*rsd
                        nc.vector.tensor_scalar(
                            out=ab, in0=ab, scalar1=musd[:, 3:4],
                            scalar2=None, op0=AOP.mult)
                        corr = ep.tile([128, 128], F32, tag="corr")
                        nc.vector.tensor_scalar(
                            out=corr[:, :w],
                            in0=sdrep[:, b * 128:b * 128 + w],
                            scalar1=rowWmu[:], scalar2=None, op0=AOP.mult)
                        nc.vector.tensor_tensor(out=ab, in0=ab,
                                                in1=corr[:, :w],
                                                op=AOP.subtract)
                    s1 = ep.tile([128, 1], F32, tag="s1")
                    nc.scalar.activation(out=ab, in_=ab, func=AF.Relu,
                                         bias=convb[:, li:li + 1], scale=1.0,
                                         accum_out=s1[:])
                    sq = ep.tile([128, 128], F32, tag="sq")
                    s2 = ep.tile([128, 1], F32, tag="s2")
                    nc.scalar.activation(out=sq[:, :w], in_=ab, func=AF.Square,
                                         bias=0.0, scale=1.0, accum_out=s2[:])
                    pend2.append((b, s1, s2))

                def stage_b():
                    nonlocal pool_ps
                    b, s1, s2 = pend2.pop(0)
                    w = 128 if b < NBLK - 1 else LASTW
                    ab = acc[:, b * 128:b * 128 + w]
                    nc.vector.tensor_tensor(out=stats[:, 0:1], in0=stats[:, 0:1],
                                            in1=s1[:], op=AOP.add)
                    nc.vector.tensor_tensor(out=stats[:, 1:2], in0=stats[:, 1:2],
                                            in1=s2[:], op=AOP.add)
                    hb16 = hpool.tile([128, 128], BF16, tag="hnorm")
                    nc.scalar.activation(out=hb16[:, :w], in_=ab, func=AF.Copy,
                                         bias=0.0, scale=1.0)
                    if li < LAYERS - 1:
                        emit_hws_block(li + 1, b, hb16[:])
                        if b in (32, 57, 82):
                            emit_allgather((b - 8) // 25)
                    else:
                        if pool_ps is None:
                            pool_ps = ppool.tile([128, NGRAPH], F32,
                                                 tag="pool")
                        ps_t = pp.tile([128, 128], BF16, tag="mm")
                        nc.tensor.transpose(out=ps_t[:], in_=hb16[:],
                                            identity=ident[:])
                        h3 = hpool.tile([128, 128], BF16, tag="h3")
                        nc.scalar.activation(out=h3[:w, :], in_=ps_t[:w, :],
                                             func=AF.Copy, bias=0.0, scale=1.0)
                        segp = hpool.tile([128, NGRAPH], BF16, tag="segp")
                        nc.vector.tensor_tensor(
                            out=segp[:w, :],
                            in0=pslot[:w, b:b + 1].to_broadcast([w, NGRAPH]),
                            in1=iota256[:w, :], op=AOP.is_equal)
                        pend3.append((b, h3, segp))

                def stage_c():
                    b, h3, segp = pend3.pop(0)
                    w = 128 if b < NBLK - 1 else LASTW
                    nc.tensor.matmul(out=pool_ps[:], lhsT=h3[:w, :],
                                     rhs=segp[:w, :],
                                     start=(b == 0), stop=(b == NBLK - 1),
                                     skip_group_check=True)

                def emit_epilogue(b):
                    stage_a(b)
                    if len(pend2) > 7:
                        stage_b()
                    if len(pend3) > 4:
                        stage_c()

                def flush_epilogues():
                    while pend2:
                        stage_b()
                    while pend3:
                        stage_c()

                gtile = 0
                open_psum = None
                open_block = -1
                open_win = -1
                open_fresh = False
                remaining = 0
                slab_tiles = {}

                def load_slab(k):
                    if k * SLABCH >= NCHUNK:
                        return
                    st = ipool.tile([128, SLABCH * CHUNK // 16], I16,
                                    tag="idxslab")
                    wsl = min(SLABCH * CHUNK, ECAP - k * SLABCH * CHUNK) // 16
                    nc.sync.dma_start(
                        out=st[:, :wsl],
                        in_=idx_in[:, k * SLABCH * CHUNK // 16:
                                   k * SLABCH * CHUNK // 16 + wsl])
                    slab_tiles[k] = st

                pending_epi = []

                def close_group():
                    nonlocal open_psum, open_block, open_win, open_fresh
                    if open_psum is None:
                        return
                    dstr = acc[:, open_block * 128:(open_block + 1) * 128]
                    if open_fresh:
                        nc.vector.tensor_copy(out=dstr, in_=open_psum[:])
                    else:
                        nc.vector.tensor_tensor(out=dstr, in0=dstr,
                                                in1=open_psum[:], op=AOP.add)
                    open_psum = None
                    if open_win == NWIN - 1:
                        pending_epi.append(open_block)
                        if len(pending_epi) > 6:
                            emit_epilogue(pending_epi.pop(0))

                load_slab(0)
                load_slab(1)
                for ch in range(NCHUNK):
                    k = ch // SLABCH
                    if ch % SLABCH == 0 and (k + 1) not in slab_tiles:
                        load_slab(k + 1)
                    idx_slab = slab_tiles[k]
                    ww = int(tile_win[ch * TPC])
                    gb = gpool.tile([128, TPC, F], BF16, tag="gb")
                    off = (ch % SLABCH) * (CHUNK // 16)
                    nc.gpsimd.dma_gather(
                        gb[:], tbl_w[ww][:],
                        idx_slab[:, off:off + CHUNK // 16],
                        CHUNK, CHUNK, F, single_packet=True,
                        queue_num=nextq())
                    gbf = gb[:].rearrange("p a b -> p (a b)")
                    for t in range(TPC):
                        b = int(tile_block[gtile])
                        w = int(tile_win[gtile])
                        if b != open_block or w != open_win:
                            close_group()
                            open_psum = ppa.tile([128, 128], F32, tag="agg")
                            open_block = b
                            open_win = w
                            open_fresh = (w == 0)
                            remaining = int(group_len[gtile])
                        seg = spool.tile([128, 128], BF16, tag="seg")
                        nc.vector.tensor_tensor(
                            out=seg[:],
                            in0=slot_t[:, gtile:gtile + 1]
                            .to_broadcast([128, 128]),
                            in1=iota128[:], op=AOP.is_equal)
                        nc.tensor.matmul(
                            out=open_psum[:], lhsT=gbf[:, t * F:(t + 1) * F],
                            rhs=seg[:],
                            start=(remaining == int(group_len[gtile])),
                            stop=(remaining == 1))
                        remaining -= 1
                        gtile += 1
                close_group()
                for b in pending_epi:
                    emit_epilogue(b)
                pending_epi = []
                flush_epilogues()
                if li < LAYERS - 1:
                    emit_allgather(3)
                open_block = -1
                open_win = -1

                # ---- LN stats all-reduce (off critical path) ----
                st_in = dram2.tile([128, 2], F32, tag=f"stin{li}")
                st_out = dram2.tile([128, 2], F32, tag=f"stout{li}")
                nc.sync.dma_start(out=st_in[:], in_=stats[:])
                nc.gpsimd.collective_compute(
                    "AllReduce", AOP.add, replica_groups=rg,
                    ins=[st_in[:]], outs=[st_out[:]])
                stg = ep.tile([128, 2], F32, tag="stg")
                nc.sync.dma_start(out=stg[:], in_=st_out[:])
                stg16 = ep.tile([128, 2], BF16, tag="stg16")
                nc.vector.tensor_copy(out=stg16[:], in_=stg[:])
                ps_s = pp.tile([1, 2], F32, tag="mm")
                nc.tensor.matmul(out=ps_s[:], lhsT=ones_col[:], rhs=stg16[:],
                                 start=True, stop=True)
                sc = ep.tile([1, 4], F32, tag="sc")
                nc.scalar.activation(out=sc[:, 0:2], in_=ps_s[:], func=AF.Copy,
                                     bias=0.0, scale=1.0 / (N_NODES * F))
                nc.vector.tensor_tensor(out=sc[:, 2:3], in0=sc[:, 0:1],
                                        in1=sc[:, 0:1], op=AOP.mult)
                nc.vector.tensor_tensor(out=sc[:, 2:3], in0=sc[:, 1:2],
                                        in1=sc[:, 2:3], op=AOP.subtract)
                nc.vector.tensor_scalar(out=sc[:, 2:3], in0=sc[:, 2:3],
                                        scalar1=EPS, scalar2=None,
                                        op0=AOP.add)
                nc.vector.reciprocal(out=sc[:, 3:4], in_=sc[:, 2:3])
                nc.scalar.activation(out=sc[:, 3:4], in_=sc[:, 3:4],
                                     func=AF.Sqrt, bias=0.0, scale=1.0)
                sc16 = ep.tile([1, 4], BF16, tag="sc16")
                nc.vector.tensor_copy(out=sc16[:], in_=sc[:])
                ps_b = pp.tile([128, 4], F32, tag="mm")
                nc.tensor.matmul(out=ps_b[:], lhsT=ones_row1[:], rhs=sc16[:],
                                 start=True, stop=True)
                musd = cst.tile([128, 4], F32, tag=f"musd{li}")
                nc.vector.tensor_copy(out=musd[:], in_=ps_b[:])

            # ---------------- pooled AllReduce + MLP head ----------------
            pooledT = work.tile([128, NGRAPH], F32, tag="pooledT")
            nc.vector.tensor_copy(out=pooledT[:], in_=pool_ps[:])
            pl_in = dram2.tile([128, NGRAPH], F32, tag="plin")
            pl_out = dram2.tile([128, NGRAPH], F32, tag="plout")
            nc.sync.dma_start(out=pl_in[:], in_=pooledT[:])
            nc.gpsimd.collective_compute(
                "AllReduce", AOP.add, replica_groups=rg,
                ins=[pl_in[:]], outs=[pl_out[:]])
            pooled = work.tile([128, NGRAPH], F32, tag="pooled2")
            nc.sync.dma_start(out=pooled[:], in_=pl_out[:])
            invcnt = work.tile([128, NGRAPH], F32, tag="invcnt")
            nc.sync.dma_start(out=invcnt[:], in_=invcntr_in[:])
            nc.vector.tensor_tensor(out=pooled[:], in0=pooled[:],
                                    in1=invcnt[:], op=AOP.mult)
            # pooled LN correction: pooled = rsd*pooled - rsd*mu
            rsdmu = work.tile([128, 1], F32, tag="rsdmu")
            nc.vector.tensor_tensor(out=rsdmu[:], in0=musd[:, 3:4],
                                    in1=musd[:, 0:1], op=AOP.mult)
            nc.vector.tensor_scalar(out=pooled[:], in0=pooled[:],
                                    scalar1=musd[:, 3:4], scalar2=rsdmu[:],
                                    op0=AOP.mult, op1=AOP.subtract)
            pooled16 = work.tile([128, NGRAPH], BF16, tag="pooled16")
            nc.vector.tensor_copy(out=pooled16[:], in_=pooled[:])

            mlpW1 = work.tile([F, F], BF16, tag="mlpW1")
            nc.sync.dma_start(out=mlpW1[:], in_=mlpW1_in[:])
            mlpb1 = work.tile([F, 1], F32, tag="mlpb1")
            nc.sync.dma_start(out=mlpb1[:], in_=mlpb1_in[:])
            mlpW2 = work.tile([F, NCLS], BF16, tag="mlpW2")
            nc.sync.dma_start(out=mlpW2[:], in_=mlpW2_in[:])
            mlpb2r = work.tile([128, NCLS], F32, tag="mlpb2r")
            nc.sync.dma_start(out=mlpb2r[:], in_=mlpb2r_in[:])

            ps_g = pp.tile([128, NGRAPH], F32, tag="mm")
            nc.tensor.matmul(out=ps_g[:], lhsT=mlpW1[:], rhs=pooled16[:],
                             start=True, stop=True)
            gT = work.tile([128, NGRAPH], BF16, tag="gT")
            nc.scalar.activation(out=gT[:], in_=ps_g[:], func=AF.Relu,
                                 bias=mlpb1[:], scale=1.0)
            for half in range(2):
                ps_sc = pp.tile([128, NCLS], F32, tag="mm")
                nc.tensor.matmul(out=ps_sc[:],
                                 lhsT=gT[:, half * 128:(half + 1) * 128],
                                 rhs=mlpW2[:], start=True, stop=True)
                scr = work.tile([128, NCLS], F32, tag="scr")
                nc.vector.tensor_tensor(out=scr[:], in0=ps_sc[:],
                                        in1=mlpb2r[:], op=AOP.add)
                mx = work.tile([128, 1], F32, tag="mx")
                nc.vector.tensor_reduce(out=mx[:], in_=scr[:],
                                        axis=mybir.AxisListType.X,
                                        op=AOP.max)
                nc.vector.tensor_scalar(out=scr[:], in0=scr[:], scalar1=mx[:],
                                        scalar2=None, op0=AOP.subtract)
                ex = work.tile([128, NCLS], F32, tag="ex")
                sm = work.tile([128, 1], F32, tag="sm")
                nc.scalar.activation(out=ex[:], in_=scr[:], func=AF.Exp,
                                     bias=0.0, scale=1.0, accum_out=sm[:])
                ls = work.tile([128, 1], F32, tag="ls")
                nc.scalar.activation(out=ls[:], in_=sm[:], func=AF.Ln,
                                     bias=0.0, scale=1.0)
                nc.vector.tensor_scalar(out=scr[:], in0=scr[:], scalar1=ls[:],
                                        scalar2=None, op0=AOP.subtract)
                nc.sync.dma_start(out=out_ext[half * 128:(half + 1) * 128, :],
                                  in_=scr[:])

    nc.compile()
    return nc


def _wrap_cols(vec, fill):
    """[NSH] -> [128, NBLK] with node b*128+p at [p, b]."""
    padded = np.full(NBLK * 128, fill, np.float32)
    padded[:NSH] = vec
    return np.ascontiguousarray(padded.reshape(NBLK, 128).T)


def _prepare(inputs):
    x = np.asarray(inputs["x"], dtype=np.float32)
    edge_index = np.asarray(inputs["edge_index"])
    batch = np.asarray(inputs["batch"], dtype=np.int64)
    assert x.shape == (N_NODES, F), x.shape

    dinv, sdplus, idxw, slotw, meta = _host_preprocess(edge_index)

    cnt = np.bincount(batch, minlength=NGRAPH).astype(np.float64)
    invcnt = (1.0 / np.maximum(cnt, 1.0)).astype(np.float32)
    iota128 = np.broadcast_to(np.arange(128, dtype=np.float32), (128, 128))
    iota256 = np.broadcast_to(np.arange(256, dtype=np.float32), (128, 256))

    lin1_W = np.asarray(inputs["lin1_W"], np.float32)
    lin1_b = np.asarray(inputs["lin1_b"], np.float32)
    conv_W = np.asarray(inputs["conv_W"], np.float32)
    conv_b = np.asarray(inputs["conv_b"], np.float32)
    mlp_W1 = np.asarray(inputs["mlp_W1"], np.float32)
    mlp_b1 = np.asarray(inputs["mlp_b1"], np.float32)
    mlp_W2 = np.asarray(inputs["mlp_W2"], np.float32)
    mlp_b2 = np.asarray(inputs["mlp_b2"], np.float32)

    convW_cat = np.concatenate([conv_W[l] for l in range(LAYERS)], axis=1)
    rowW = np.stack([conv_W[l].sum(axis=0) for l in range(LAYERS)],
                    axis=1)  # [F, LAYERS]

    in_maps = []
    for c in range(NCORES):
        lo, hi = c * NSH, (c + 1) * NSH
        xT = np.zeros((F, NBLK * 128), np.float32)
        xT[:, :NSH] = x[lo:hi].T
        xT = xT.astype(BF)
        dinv_pad = np.zeros(NBLK * 128, np.float32)
        dinv_pad[:NSH] = dinv[lo:hi]
        sd_pad = np.zeros(NBLK * 128, np.float32)
        sd_pad[:NSH] = (sdplus * dinv)[lo:hi]
        in_maps.append({
            "xT": xT,
            "idx": idxw[c],
            "slot": slotw[c],
            "dinvrep": np.ascontiguousarray(
                np.broadcast_to(dinv_pad, (128, NBLK * 128))).astype(BF),
            "sdrep": np.ascontiguousarray(
                np.broadcast_to(sd_pad, (128, NBLK * 128))).astype(BF),
            "dinvw": _wrap_cols(dinv[lo:hi], 0.0),
            "pslot": _wrap_cols(batch[lo:hi].astype(np.float32),
                                300.0).astype(BF),
            "iota128": iota128.astype(BF),
            "iota4": np.ascontiguousarray(
                np.broadcast_to(np.tile(np.arange(128, dtype=np.float32), 4),
                                (128, 512))).astype(BF),
            "iota256": iota256.astype(BF),
            "lin1W": lin1_W.astype(BF),
            "lin1b": np.ascontiguousarray(lin1_b.reshape(F, 1)),
            "convW": convW_cat.astype(BF),
            "convb": np.ascontiguousarray(conv_b.T),
            "rowW": np.ascontiguousarray(rowW),
            "mlpW1": mlp_W1.astype(BF),
            "mlpb1": np.ascontiguousarray(mlp_b1.reshape(F, 1)),
            "mlpW2": mlp_W2.astype(BF),
            "mlpb2r": np.ascontiguousarray(
                np.broadcast_to(mlp_b2, (128, NCLS)).astype(np.float32)),
            "invcntr": np.ascontiguousarray(
                np.broadcast_to(invcnt, (128, NGRAPH))),
        })
    return meta, in_maps


_CACHED = {}


def kernel_run(inputs, trace=False):
    meta, in_maps = _prepare(inputs)
    key = meta["TT"]
    if key not in _CACHED:
        _CACHED[key] = _build_program(meta)
    nc = _CACHED[key]
    res = run_bass_kernel_spmd(nc, in_maps, core_ids=list(range(NCORES)),
                               trace=trace)
    out = np.asarray(res.results[0]["out"], dtype=np.float32)
    return out, res.exec_time_ns


def kernel(**inputs):
    out, _ = kernel_run(inputs, trace=False)
    return out


# revision 30
# speedup vs baseline: 1.0720x; 1.0101x over previous
"""Trainium2 Bass kernel for the GCN message-passing model (8 NeuronCores).

Strategy (v2)
-------------
- Nodes (and their incoming edges) are sharded by destination across 8 cores
  (12500 nodes each).  Self-loops join the gather stream as ordinary edges:
  with hws_pre already carrying dinv[src], the epilogue's dinv[d] factor
  yields the exact dinv^2[d]*(h@W)[d] self contribution.
- LayerNorm is folded forward algebraically: since aggregation is linear and
  LN is affine (h_norm = rsd*(h - mu)), each layer gathers UN-normalized
  rows hws_pre = dinv*(relu_h @ W) and the epilogue applies
      out = rsd * (acc_raw - mu * rowW[f] * SD+[d]) * dinv[d]
  where rowW = colsum(W) and SD+[d] = sum_{s in N(d)} dinv[s] + dinv[d]^2 is
  host-precomputed.  The LN stats AllReduce therefore never blocks the
  gather pipeline.
- The per-layer AllGather of hws_pre is split into 4 window slices (blocks
  0-24 / 25-49 / 50-74 / 75-97 of every core) so window w+1's collective
  overlaps window w's edge gathering.
- Edge aggregation: edges bucketed per (dst-block, src-window); dma_gather
  (1024 idx/chunk, 4 queues, queue = chunk index mod 4) fetches bf16 rows;
  per 128-edge tile a DVE is_equal one-hot + TensorE matmul accumulates
  feature-major per-block sums in PSUM, then adds into the SBUF accumulator.
- Graph mean pooling reuses the segment-matmul trick on sorted `batch`
  (also LN-folded: pool relu_h, correct with scalars), then an AllReduce and
  a small replicated MLP head + log_softmax.
"""

import sys

sys.path.insert(0, "/opt/trn_rl_repo")

import numpy as np
import ml_dtypes

import concourse.bass as bass
import concourse.bacc as bacc
import concourse.mybir as mybir
import concourse.tile as tile
from concourse.bass_utils import run_bass_kernel_spmd
from concourse.library_config import mlp as gpsimd_mlp_lib
from concourse.masks import make_identity

NCORES = 8
N_NODES = 100_000
F = 128          # feature/hidden width
NCLS = 10
LAYERS = 3
NGRAPH = 256
EPS = 1e-5
NSH = N_NODES // NCORES          # 12500 nodes per core
NBLK = (NSH + 127) // 128        # 98 blocks of 128 dst rows
LASTW = NSH - (NBLK - 1) * 128   # 84 rows in the last block
NWIN = 4
WBLK = [25, 25, 25, 23]          # blocks per window
WSZ = [3200, 3200, 3200, 2900]   # rows per window per core
WOFF = [0, 3200, 6400, 9600]
WTOK = [sz * NCORES for sz in WSZ]
CHUNK = 1024                     # idxs per dma_gather (hard HW limit)
TPC = CHUNK // 128               # tiles per chunk
NQUEUE = 4
SLABCH = 16                      # gather chunks per idx slab load

BF16 = mybir.dt.bfloat16
F32 = mybir.dt.float32
I16 = mybir.dt.int16
AOP = mybir.AluOpType
AF = mybir.ActivationFunctionType
BF = ml_dtypes.bfloat16


def _host_preprocess(edge_index):
    """Per-core edge tiling + gather indices (self-loops excluded)."""
    src = np.asarray(edge_index[0], dtype=np.int64)
    dst = np.asarray(edge_index[1], dtype=np.int64)
    deg = np.bincount(dst, minlength=N_NODES).astype(np.float64) + 1.0
    dinv = (1.0 / np.sqrt(deg)).astype(np.float32)
    # +I self-loops join the gather stream as ordinary edges: with hws_pre
    # already carrying dinv[src], the epilogue's dinv[d] factor makes the
    # self contribution dinv^2[d]*(h@W)[d] exactly.
    loop = np.arange(N_NODES, dtype=np.int64)
    src = np.concatenate([src, loop])
    dst = np.concatenate([dst, loop])

    # SD+[d] = sum_{s in N(d) incl loop} dinv[s]  (for the LN correction)
    sdp = np.zeros(N_NODES, np.float64)
    np.add.at(sdp, dst, dinv[src].astype(np.float64))
    sdplus = sdp.astype(np.float32)

    core = dst // NSH
    blk = (dst % NSH) // 128
    slot = (dst % NSH) % 128
    srcr = src % NSH
    win = np.minimum(srcr // 3200, 3)
    tok = (src // NSH) * np.array(WSZ)[win] + (srcr - np.array(WOFF)[win])

    key = (core * NBLK + blk) * NWIN + win
    order = np.lexsort((src, key))
    key_s = key[order]
    tok_s = tok[order]
    slot_s = slot[order]
    ngroups = NCORES * NBLK * NWIN
    counts = np.bincount(key_s, minlength=ngroups).reshape(NCORES, NBLK, NWIN)
    starts = np.zeros(ngroups + 1, dtype=np.int64)
    np.cumsum(counts.reshape(-1), out=starts[1:])

    # uniform tile grid: T[b][w] = max over cores of ceil(count/128)
    T = np.maximum((counts + 127) // 128, 1).max(axis=0)  # [NBLK, NWIN]
    for w in range(NWIN):
        T[NBLK - 1, w] += (-int(T[:, w].sum())) % TPC
    TT = int(T.sum())
    ecap = TT * 128

    tile_block = np.empty(TT, dtype=np.int64)
    tile_win = np.empty(TT, dtype=np.int64)
    group_len = np.empty(TT, dtype=np.int64)
    t0 = 0
    for w in range(NWIN):
        for b in range(NBLK):
            n = int(T[b, w])
            tile_block[t0:t0 + n] = b
            tile_win[t0:t0 + n] = w
            group_len[t0:t0 + n] = n
            t0 += n
    assert t0 == TT

    idx16 = np.zeros((NCORES, ecap), dtype=np.int16)
    slots = np.full((NCORES, ecap), 255, dtype=np.float32)
    for c in range(NCORES):
        pos = 0
        for w in range(NWIN):
            for b in range(NBLK):
                g = (c * NBLK + b) * NWIN + w
                s0, s1 = starts[g], starts[g + 1]
                n = s1 - s0
                idx16[c, pos:pos + n] = tok_s[s0:s1].astype(np.int16)
                slots[c, pos:pos + n] = slot_s[s0:s1].astype(np.float32)
                pos += int(T[b, w]) * 128
        assert pos == ecap

    idxw = idx16.reshape(NCORES, -1, 16).transpose(0, 2, 1)
    idxw = np.ascontiguousarray(np.tile(idxw, (1, 8, 1)))
    slotw = np.ascontiguousarray(
        slots.reshape(NCORES, TT, 128).transpose(0, 2, 1)).astype(BF)

    meta = dict(TT=TT, tile_block=tile_block, tile_win=tile_win,
                group_len=group_len)
    return dinv, sdplus, idxw, slotw, meta


def _build_program(meta):
    """Trace the SPMD Bass/Tile program (shared by all 8 cores)."""
    TT = meta["TT"]
    tile_block = meta["tile_block"]
    tile_win = meta["tile_win"]
    group_len = meta["group_len"]
    ECAP = TT * 128
    NCHUNK = ECAP // CHUNK
    # first chunk of each window
    win_chunk0 = [int(np.searchsorted(tile_win, w)) // TPC for w in range(NWIN)]

    nc = bacc.Bacc("TRN2", target_bir_lowering=False, debug=False,
                   num_devices=NCORES, num_swdge_queues=NQUEUE)

    # ---- external inputs (per core) ----
    xT_in = nc.declare_dram_parameter("xT", [F, NBLK * 128], BF16, isOutput=False)
    idx_in = nc.declare_dram_parameter("idx", [128, ECAP // 16], I16, isOutput=False)
    slot_in = nc.declare_dram_parameter("slot", [128, TT], BF16, isOutput=False)
    dinvrep_in = nc.declare_dram_parameter("dinvrep", [128, NBLK * 128], BF16, isOutput=False)
    sdrep_in = nc.declare_dram_parameter("sdrep", [128, NBLK * 128], BF16, isOutput=False)
    dinvw_in = nc.declare_dram_parameter("dinvw", [128, NBLK], F32, isOutput=False)
    pslot_in = nc.declare_dram_parameter("pslot", [128, NBLK], BF16, isOutput=False)
    iota128_in = nc.declare_dram_parameter("iota128", [128, 128], BF16, isOutput=False)
    iota4_in = nc.declare_dram_parameter("iota4", [128, 512], BF16, isOutput=False)
    iota256_in = nc.declare_dram_parameter("iota256", [128, 256], BF16, isOutput=False)
    lin1W_in = nc.declare_dram_parameter("lin1W", [F, F], BF16, isOutput=False)
    lin1b_in = nc.declare_dram_parameter("lin1b", [F, 1], F32, isOutput=False)
    convW_in = nc.declare_dram_parameter("convW", [F, LAYERS * F], BF16, isOutput=False)
    convb_in = nc.declare_dram_parameter("convb", [F, LAYERS], F32, isOutput=False)
    rowW_in = nc.declare_dram_parameter("rowW", [F, LAYERS], F32, isOutput=False)
    mlpW1_in = nc.declare_dram_parameter("mlpW1", [F, F], BF16, isOutput=False)
    mlpb1_in = nc.declare_dram_parameter("mlpb1", [F, 1], F32, isOutput=False)
    mlpW2_in = nc.declare_dram_parameter("mlpW2", [F, NCLS], BF16, isOutput=False)
    mlpb2r_in = nc.declare_dram_parameter("mlpb2r", [128, NCLS], F32, isOutput=False)
    invcntr_in = nc.declare_dram_parameter("invcntr", [128, NGRAPH], F32, isOutput=False)
    out_ext = nc.declare_dram_parameter("out", [NGRAPH, NCLS], F32, isOutput=True)

    rg = [list(range(NCORES))]

    with tile.TileContext(nc) as tc:
        with tc.tile_pool(name="const", bufs=1) as cst, \
             tc.tile_pool(name="big", bufs=1) as big, \
             tc.tile_pool(name="work", bufs=8) as work, \
             tc.tile_pool(name="epil", bufs=8) as ep, \
             tc.tile_pool(name="segp", bufs=24) as spool, \
             tc.tile_pool(name="hxp", bufs=6) as hpool, \
             tc.tile_pool(name="gbuf", bufs=12) as gpool, \
             tc.tile_pool(name="idxs", bufs=2) as ipool, \
             tc.tile_pool(name="psum", bufs=3, space="PSUM") as pp, \
             tc.tile_pool(name="psag", bufs=4, space="PSUM") as ppa, \
             tc.tile_pool(name="ppool", bufs=1, space="PSUM") as ppool, \
             tc.tile_pool(name="dram", bufs=1, space="DRAM") as dram, \
             tc.tile_pool(name="dram2", bufs=1, space="DRAM") as dram2:

            nc.gpsimd.load_library(gpsimd_mlp_lib)

            # ---- persistent SBUF constants ----
            slot_t = cst.tile([128, TT], BF16)
            nc.sync.dma_start(out=slot_t[:], in_=slot_in[:])
            dinvrep = cst.tile([128, NBLK * 128], BF16)
            nc.sync.dma_start(out=dinvrep[:], in_=dinvrep_in[:])
            sdrep = cst.tile([128, NBLK * 128], BF16)
            nc.sync.dma_start(out=sdrep[:], in_=sdrep_in[:])
            dinvw = cst.tile([128, NBLK], F32)
            nc.sync.dma_start(out=dinvw[:], in_=dinvw_in[:])
            pslot = cst.tile([128, NBLK], BF16)
            nc.sync.dma_start(out=pslot[:], in_=pslot_in[:])
            iota128 = cst.tile([128, 128], BF16)
            nc.sync.dma_start(out=iota128[:], in_=iota128_in[:])
            iota4 = cst.tile([128, 512], BF16)
            nc.sync.dma_start(out=iota4[:], in_=iota4_in[:])
            iota256 = cst.tile([128, 256], BF16)
            nc.sync.dma_start(out=iota256[:], in_=iota256_in[:])
            lin1W = cst.tile([F, F], BF16)
            nc.sync.dma_start(out=lin1W[:], in_=lin1W_in[:])
            lin1b = cst.tile([F, 1], F32)
            nc.sync.dma_start(out=lin1b[:], in_=lin1b_in[:])
            convW = cst.tile([F, LAYERS * F], BF16)
            nc.sync.dma_start(out=convW[:], in_=convW_in[:])
            convb = cst.tile([F, LAYERS], F32)
            nc.sync.dma_start(out=convb[:], in_=convb_in[:])
            rowW = cst.tile([F, LAYERS], F32)
            nc.sync.dma_start(out=rowW[:], in_=rowW_in[:])
            ones_col = cst.tile([128, 1], BF16)
            nc.vector.memset(ones_col[:], 1.0)
            ones_row1 = cst.tile([1, 128], BF16)
            nc.vector.memset(ones_row1[:], 1.0)
            ident = cst.tile([128, 128], BF16)
            make_identity(nc, ident[:])

            # feature-major accumulator
            acc = big.tile([128, NBLK * 128], F32)

            for _ in range(12):
                g0 = gpool.tile([128, TPC, F], BF16, tag="gb")
                nc.vector.memset(g0[:].rearrange("p a b -> p (a b)"), 0.0)

            # DRAM: per-window shard slices + gathered tables
            hws_shard_w = [dram.tile([WSZ[w], F], BF16, name=f"shardw{w}",
                                     tag=f"shard{w}") for w in range(NWIN)]
            tbl_w = [dram.tile([WTOK[w], F], BF16, name=f"tblw{w}",
                               tag=f"tbl{w}") for w in range(NWIN)]

            qctr = [0]

            def nextq():
                q = qctr[0] % NQUEUE
                qctr[0] += 1
                return q

            # ---------------- helpers ----------------
            def emit_hws_block(i, b, lhsT_fm):
                """hws_pre block b for layer i: psum = lhsT_fm.T@W_i, *dinv,
                store to hws_self + DRAM window slice; returns nothing."""
                W = convW[:, i * F:(i + 1) * F]
                w = 128 if b < NBLK - 1 else LASTW
                ps = pp.tile([128, F], F32, tag="mm")
                nc.tensor.matmul(out=ps[:w, :], lhsT=lhsT_fm[:, :w], rhs=W,
                                 start=True, stop=True)
                hb = hpool.tile([128, F], BF16, tag="hws")
                nc.vector.tensor_scalar(
                    out=hb[:w, :], in0=ps[:w, :],
                    scalar1=dinvw[:w, b:b + 1], scalar2=None, op0=AOP.mult)
                wi = min(b // 25, 3)
                r0 = b * 128 - WOFF[wi]
                nc.sync.dma_start(out=hws_shard_w[wi][r0:r0 + w, :],
                                  in_=hb[:w, :])

            def emit_allgather(wi):
                nc.gpsimd.collective_compute(
                    "AllGather", AOP.bypass, replica_groups=rg,
                    ins=[hws_shard_w[wi][:]], outs=[tbl_w[wi][:]])

            # hws_self stores node-major [slot, f] per block: hws_self[:, b*128+f]?
            # Layout: hws_self[p, b*128 + f] = hws_pre[node b*128+p, f]
            # (partition = slot, block-major along free).  emit_hws_block wrote
            # hb [slot, f] into hws_self[:, b*128 : b*128+128]... but widths:
            # block stride along free must be 128 (f), so hws_self is
            # [128 slots, NBLK * F].  (LASTW rows: unused slots hold garbage,
            # killed by diag column zeros.)

            # ---------------- P0: h1 = relu(x@lin1+b); hws_pre^0 ----------------
            for b in range(NBLK):
                xb = work.tile([128, 128], BF16, tag="xb")
                nc.sync.dma_start(out=xb[:],
                                  in_=xT_in[:, b * 128:(b + 1) * 128])
                ps = pp.tile([128, 128], F32, tag="mm")
                nc.tensor.matmul(out=ps[:], lhsT=lin1W[:], rhs=xb[:],
                                 start=True, stop=True)
                h1b = work.tile([128, 128], BF16, tag="h1")
                nc.scalar.activation(out=h1b[:], in_=ps[:], func=AF.Relu,
                                     bias=lin1b[:], scale=1.0)
                emit_hws_block(0, b, h1b[:])
                if b in (24, 49, 74, 97):
                    emit_allgather(min(b // 25, 3))

            # ---------------- conv layers ----------------
            pool_ps = None
            musd = None
            for li in range(LAYERS):
                # ---- pass 1: edge aggregation into `acc` (feature-major) ----
                stats = cst.tile([128, 2], F32, tag=f"stats{li}")
                nc.vector.memset(stats[:], 0.0)

                # per-layer LN-correction scalars (from layer li-1 stats)
                if li > 0:
                    rowWmu = ep.tile([128, 1], F32, tag="rowWmu")
                    nc.vector.tensor_scalar(
                        out=rowWmu[:], in0=rowW[:, li:li + 1],
                        scalar1=musd[:, 0:1], scalar2=musd[:, 3:4],
                        op0=AOP.mult, op1=AOP.mult)

                # Epilogue is a 3-stage software pipeline over closed blocks:
                # each stage is emitted several block-closes after the one
                # producing its inputs, so no in-order engine stream ever
                # blocks on a cross-engine dependency (which would starve the
                # gather loop).
                pend2 = []   # after stage A: (b, s1, s2)
                pend3 = []   # after stage B (li==2 pooling): (b, h3, segp)

                def stage_a(b):
                    """DVE corrections (acc-local) + Scalar relu/square."""
                    w = 128 if b < NBLK - 1 else LASTW
                    ab = acc[:, b * 128:b * 128 + w]
                    nc.vector.tensor_tensor(
                        out=ab, in0=ab,
                        in1=dinvrep[:, b * 128:b * 128 + w], op=AOP.mult)
                    if li > 0:
                        # z = rsd*(acc*dinv) - corr, corr folded with mu*rsd
                        nc.vector.tensor_scalar(
                            out=ab, in0=ab, scalar1=musd[:, 3:4],
                            scalar2=None, op0=AOP.mult)
                        corr = ep.tile([128, 128], F32, tag="corr")
                        nc.vector.tensor_scalar(
                            out=corr[:, :w],
                            in0=sdrep[:, b * 128:b * 128 + w],
                            scalar1=rowWmu[:], scalar2=None, op0=AOP.mult)
                        nc.vector.tensor_tensor(out=ab, in0=ab,
                                                in1=corr[:, :w],
                                                op=AOP.subtract)
                    s1 = ep.tile([128, 1], F32, tag="s1")
                    nc.scalar.activation(out=ab, in_=ab, func=AF.Relu,
                                         bias=convb[:, li:li + 1], scale=1.0,
                                         accum_out=s1[:])
                    sq = ep.tile([128, 128], F32, tag="sq")
                    s2 = ep.tile([128, 1], F32, tag="s2")
                    nc.scalar.activation(out=sq[:, :w], in_=ab, func=AF.Square,
                                         bias=0.0, scale=1.0, accum_out=s2[:])
                    pend2.append((b, s1, s2))

                def stage_b():
                    nonlocal pool_ps
                    b, s1, s2 = pend2.pop(0)
                    w = 128 if b < NBLK - 1 else LASTW
                    ab = acc[:, b * 128:b * 128 + w]
                    nc.vector.tensor_tensor(out=stats[:, 0:1], in0=stats[:, 0:1],
                                            in1=s1[:], op=AOP.add)
                    nc.vector.tensor_tensor(out=stats[:, 1:2], in0=stats[:, 1:2],
                                            in1=s2[:], op=AOP.add)
                    hb16 = hpool.tile([128, 128], BF16, tag="hnorm")
                    nc.scalar.activation(out=hb16[:, :w], in_=ab, func=AF.Copy,
                                         bias=0.0, scale=1.0)
                    if li < LAYERS - 1:
                        emit_hws_block(li + 1, b, hb16[:])
                        if b in (32, 57, 82):
                            emit_allgather((b - 8) // 25)
                    else:
                        if pool_ps is None:
                            pool_ps = ppool.tile([128, NGRAPH], F32,
                                                 tag="pool")
                        ps_t = pp.tile([128, 128], BF16, tag="mm")
                        nc.tensor.transpose(out=ps_t[:], in_=hb16[:],
                                            identity=ident[:])
                        h3 = hpool.tile([128, 128], BF16, tag="h3")
                        nc.scalar.activation(out=h3[:w, :], in_=ps_t[:w, :],
                                             func=AF.Copy, bias=0.0, scale=1.0)
                        segp = hpool.tile([128, NGRAPH], BF16, tag="segp")
                        nc.vector.tensor_tensor(
                            out=segp[:w, :],
                            in0=pslot[:w, b:b + 1].to_broadcast([w, NGRAPH]),
                            in1=iota256[:w, :], op=AOP.is_equal)
                        pend3.append((b, h3, segp))

                def stage_c():
                    b, h3, segp = pend3.pop(0)
                    w = 128 if b < NBLK - 1 else LASTW
                    nc.tensor.matmul(out=pool_ps[:], lhsT=h3[:w, :],
                                     rhs=segp[:w, :],
                                     start=(b == 0), stop=(b == NBLK - 1),
                                     skip_group_check=True)

                def emit_epilogue(b):
                    stage_a(b)
                    if len(pend2) > 7:
                        stage_b()
                    if len(pend3) > 4:
                        stage_c()

                def flush_epilogues():
                    while pend2:
                        stage_b()
                    while pend3:
                        stage_c()

                gtile = 0
                open_psum = None
                open_block = -1
                open_win = -1
                open_fresh = False
                remaining = 0
                slab_tiles = {}

                def load_slab(k):
                    if k * SLABCH >= NCHUNK:
                        return
                    st = ipool.tile([128, SLABCH * CHUNK // 16], I16,
                                    tag="idxslab")
                    wsl = min(SLABCH * CHUNK, ECAP - k * SLABCH * CHUNK) // 16
                    nc.sync.dma_start(
                        out=st[:, :wsl],
                        in_=idx_in[:, k * SLABCH * CHUNK // 16:
                                   k * SLABCH * CHUNK // 16 + wsl])
                    slab_tiles[k] = st

                pending_epi = []

                def close_group():
                    nonlocal open_psum, open_block, open_win, open_fresh
                    if open_psum is None:
                        return
                    dstr = acc[:, open_block * 128:(open_block + 1) * 128]
                    if open_fresh:
                        nc.vector.tensor_copy(out=dstr, in_=open_psum[:])
                    else:
                        nc.vector.tensor_tensor(out=dstr, in0=dstr,
                                                in1=open_psum[:], op=AOP.add)
                    open_psum = None
                    if open_win == NWIN - 1:
                        pending_epi.append(open_block)
                        if len(pending_epi) > 12:
                            emit_epilogue(pending_epi.pop(0))

                load_slab(0)
                load_slab(1)
                for ch in range(NCHUNK):
                    k = ch // SLABCH
                    if ch % SLABCH == 0 and (k + 1) not in slab_tiles:
                        load_slab(k + 1)
                    idx_slab = slab_tiles[k]
                    ww = int(tile_win[ch * TPC])
                    gb = gpool.tile([128, TPC, F], BF16, tag="gb")
                    off = (ch % SLABCH) * (CHUNK // 16)
                    nc.gpsimd.dma_gather(
                        gb[:], tbl_w[ww][:],
                        idx_slab[:, off:off + CHUNK // 16],
                        CHUNK, CHUNK, F, single_packet=True,
                        queue_num=nextq())
                    gbf = gb[:].rearrange("p a b -> p (a b)")
                    for t in range(TPC):
                        b = int(tile_block[gtile])
                        w = int(tile_win[gtile])
                        if b != open_block or w != open_win:
                            close_group()
                            open_psum = ppa.tile([128, 128], F32, tag="agg")
                            open_block = b
                            open_win = w
                            open_fresh = (w == 0)
                            remaining = int(group_len[gtile])
                        seg = spool.tile([128, 128], BF16, tag="seg")
                        nc.vector.tensor_tensor(
                            out=seg[:],
                            in0=slot_t[:, gtile:gtile + 1]
                            .to_broadcast([128, 128]),
                            in1=iota128[:], op=AOP.is_equal)
                        nc.tensor.matmul(
                            out=open_psum[:], lhsT=gbf[:, t * F:(t + 1) * F],
                            rhs=seg[:],
                            start=(remaining == int(group_len[gtile])),
                            stop=(remaining == 1))
                        remaining -= 1
                        gtile += 1
                close_group()
                for b in pending_epi:
                    emit_epilogue(b)
                pending_epi = []
                flush_epilogues()
                if li < LAYERS - 1:
                    emit_allgather(3)
                open_block = -1
                open_win = -1

                # ---- LN stats all-reduce (off critical path) ----
                st_in = dram2.tile([128, 2], F32, tag=f"stin{li}")
                st_out = dram2.tile([128, 2], F32, tag=f"stout{li}")
                nc.sync.dma_start(out=st_in[:], in_=stats[:])
                nc.gpsimd.collective_compute(
                    "AllReduce", AOP.add, replica_groups=rg,
                    ins=[st_in[:]], outs=[st_out[:]])
                stg = ep.tile([128, 2], F32, tag="stg")
                nc.sync.dma_start(out=stg[:], in_=st_out[:])
                stg16 = ep.tile([128, 2], BF16, tag="stg16")
                nc.vector.tensor_copy(out=stg16[:], in_=stg[:])
                ps_s = pp.tile([1, 2], F32, tag="mm")
                nc.tensor.matmul(out=ps_s[:], lhsT=ones_col[:], rhs=stg16[:],
                                 start=True, stop=True)
                sc = ep.tile([1, 4], F32, tag="sc")
                nc.scalar.activation(out=sc[:, 0:2], in_=ps_s[:], func=AF.Copy,
                                     bias=0.0, scale=1.0 / (N_NODES * F))
                nc.vector.tensor_tensor(out=sc[:, 2:3], in0=sc[:, 0:1],
                                        in1=sc[:, 0:1], op=AOP.mult)
                nc.vector.tensor_tensor(out=sc[:, 2:3], in0=sc[:, 1:2],
                                        in1=sc[:, 2:3], op=AOP.subtract)
                nc.vector.tensor_scalar(out=sc[:, 2:3], in0=sc[:, 2:3],
                                        scalar1=EPS, scalar2=None,
                                        op0=AOP.add)
                nc.vector.reciprocal(out=sc[:, 3:4], in_=sc[:, 2:3])
                nc.scalar.activation(out=sc[:, 3:4], in_=sc[:, 3:4],
                                     func=AF.Sqrt, bias=0.0, scale=1.0)
                sc16 = ep.tile([1, 4], BF16, tag="sc16")
                nc.vector.tensor_copy(out=sc16[:], in_=sc[:])
                ps_b = pp.tile([128, 4], F32, tag="mm")
                nc.tensor.matmul(out=ps_b[:], lhsT=ones_row1[:], rhs=sc16[:],
                                 start=True, stop=True)
                musd = cst.tile([128, 4], F32, tag=f"musd{li}")
                nc.vector.tensor_copy(out=musd[:], in_=ps_b[:])

            # ---------------- pooled AllReduce + MLP head ----------------
            pooledT = work.tile([128, NGRAPH], F32, tag="pooledT")
            nc.vector.tensor_copy(out=pooledT[:], in_=pool_ps[:])
            pl_in = dram2.tile([128, NGRAPH], F32, tag="plin")
            pl_out = dram2.tile([128, NGRAPH], F32, tag="plout")
            nc.sync.dma_start(out=pl_in[:], in_=pooledT[:])
            nc.gpsimd.collective_compute(
                "AllReduce", AOP.add, replica_groups=rg,
                ins=[pl_in[:]], outs=[pl_out[:]])
            pooled = work.tile([128, NGRAPH], F32, tag="pooled2")
            nc.sync.dma_start(out=pooled[:], in_=pl_out[:])
            invcnt = work.tile([128, NGRAPH], F32, tag="invcnt")
            nc.sync.dma_start(out=invcnt[:], in_=invcntr_in[:])
            nc.vector.tensor_tensor(out=pooled[:], in0=pooled[:],
                                    in1=invcnt[:], op=AOP.mult)
            # pooled LN correction: pooled = rsd*pooled - rsd*mu
            rsdmu = work.tile([128, 1], F32, tag="rsdmu")
            nc.vector.tensor_tensor(out=rsdmu[:], in0=musd[:, 3:4],
                                    in1=musd[:, 0:1], op=AOP.mult)
            nc.vector.tensor_scalar(out=pooled[:], in0=pooled[:],
                                    scalar1=musd[:, 3:4], scalar2=rsdmu[:],
                                    op0=AOP.mult, op1=AOP.subtract)
            pooled16 = work.tile([128, NGRAPH], BF16, tag="pooled16")
            nc.vector.tensor_copy(out=pooled16[:], in_=pooled[:])

            mlpW1 = work.tile([F, F], BF16, tag="mlpW1")
            nc.sync.dma_start(out=mlpW1[:], in_=mlpW1_in[:])
            mlpb1 = work.tile([F, 1], F32, tag="mlpb1")
            nc.sync.dma_start(out=mlpb1[:], in_=mlpb1_in[:])
            mlpW2 = work.tile([F, NCLS], BF16, tag="mlpW2")
            nc.sync.dma_start(out=mlpW2[:], in_=mlpW2_in[:])
            mlpb2r = work.tile([128, NCLS], F32, tag="mlpb2r")
            nc.sync.dma_start(out=mlpb2r[:], in_=mlpb2r_in[:])

            ps_g = pp.tile([128, NGRAPH], F32, tag="mm")
            nc.tensor.matmul(out=ps_g[:], lhsT=mlpW1[:], rhs=pooled16[:],
                             start=True, stop=True)
            gT = work.tile([128, NGRAPH], BF16, tag="gT")
            nc.scalar.activation(out=gT[:], in_=ps_g[:], func=AF.Relu,
                                 bias=mlpb1[:], scale=1.0)
            for half in range(2):
                ps_sc = pp.tile([128, NCLS], F32, tag="mm")
                nc.tensor.matmul(out=ps_sc[:],
                                 lhsT=gT[:, half * 128:(half + 1) * 128],
                                 rhs=mlpW2[:], start=True, stop=True)
                scr = work.tile([128, NCLS], F32, tag="scr")
                nc.vector.tensor_tensor(out=scr[:], in0=ps_sc[:],
                                        in1=mlpb2r[:], op=AOP.add)
                mx = work.tile([128, 1], F32, tag="mx")
                nc.vector.tensor_reduce(out=mx[:], in_=scr[:],
                                        axis=mybir.AxisListType.X,
                                        op=AOP.max)
                nc.vector.tensor_scalar(out=scr[:], in0=scr[:], scalar1=mx[:],
                                        scalar2=None, op0=AOP.subtract)
                ex = work.tile([128, NCLS], F32, tag="ex")
                sm = work.tile([128, 1], F32, tag="sm")
                nc.scalar.activation(out=ex[:], in_=scr[:], func=AF.Exp,
                                     bias=0.0, scale=1.0, accum_out=sm[:])
                ls = work.tile([128, 1], F32, tag="ls")
                nc.scalar.activation(out=ls[:], in_=sm[:], func=AF.Ln,
                                     bias=0.0, scale=1.0)
                nc.vector.tensor_scalar(out=scr[:], in0=scr[:], scalar1=ls[:],
                                        scalar2=None, op0=AOP.subtract)
                nc.sync.dma_start(out=out_ext[half * 128:(half + 1) * 128, :],
                                  in_=scr[:])

    nc.compile()
    return nc


def _wrap_cols(vec, fill):
    """[NSH] -> [128, NBLK] with node b*128+p at [p, b]."""
    padded = np.full(NBLK * 128, fill, np.float32)
    padded[:NSH] = vec
    return np.ascontiguousarray(padded.reshape(NBLK, 128).T)


def _prepare(inputs):
    x = np.asarray(inputs["x"], dtype=np.float32)
    edge_index = np.asarray(inputs["edge_index"])
    batch = np.asarray(inputs["batch"], dtype=np.int64)
    assert x.shape == (N_NODES, F), x.shape

    dinv, sdplus, idxw, slotw, meta = _host_preprocess(edge_index)

    cnt = np.bincount(batch, minlength=NGRAPH).astype(np.float64)
    invcnt = (1.0 / np.maximum(cnt, 1.0)).astype(np.float32)
    iota128 = np.broadcast_to(np.arange(128, dtype=np.float32), (128, 128))
    iota256 = np.broadcast_to(np.arange(256, dtype=np.float32), (128, 256))

    lin1_W = np.asarray(inputs["lin1_W"], np.float32)
    lin1_b = np.asarray(inputs["lin1_b"], np.float32)
    conv_W = np.asarray(inputs["conv_W"], np.float32)
    conv_b = np.asarray(inputs["conv_b"], np.float32)
    mlp_W1 = np.asarray(inputs["mlp_W1"], np.float32)
    mlp_b1 = np.asarray(inputs["mlp_b1"], np.float32)
    mlp_W2 = np.asarray(inputs["mlp_W2"], np.float32)
    mlp_b2 = np.asarray(inputs["mlp_b2"], np.float32)

    convW_cat = np.concatenate([conv_W[l] for l in range(LAYERS)], axis=1)
    rowW = np.stack([conv_W[l].sum(axis=0) for l in range(LAYERS)],
                    axis=1)  # [F, LAYERS]

    in_maps = []
    for c in range(NCORES):
        lo, hi = c * NSH, (c + 1) * NSH
        xT = np.zeros((F, NBLK * 128), np.float32)
        xT[:, :NSH] = x[lo:hi].T
        xT = xT.astype(BF)
        dinv_pad = np.zeros(NBLK * 128, np.float32)
        dinv_pad[:NSH] = dinv[lo:hi]
        sd_pad = np.zeros(NBLK * 128, np.float32)
        sd_pad[:NSH] = (sdplus * dinv)[lo:hi]
        in_maps.append({
            "xT": xT,
            "idx": idxw[c],
            "slot": slotw[c],
            "dinvrep": np.ascontiguousarray(
                np.broadcast_to(dinv_pad, (128, NBLK * 128))).astype(BF),
            "sdrep": np.ascontiguousarray(
                np.broadcast_to(sd_pad, (128, NBLK * 128))).astype(BF),
            "dinvw": _wrap_cols(dinv[lo:hi], 0.0),
            "pslot": _wrap_cols(batch[lo:hi].astype(np.float32),
                                300.0).astype(BF),
            "iota128": iota128.astype(BF),
            "iota4": np.ascontiguousarray(
                np.broadcast_to(np.tile(np.arange(128, dtype=np.float32), 4),
                                (128, 512))).astype(BF),
            "iota256": iota256.astype(BF),
            "lin1W": lin1_W.astype(BF),
            "lin1b": np.ascontiguousarray(lin1_b.reshape(F, 1)),
            "convW": convW_cat.astype(BF),
            "convb": np.ascontiguousarray(conv_b.T),
            "rowW": np.ascontiguousarray(rowW),
            "mlpW1": mlp_W1.astype(BF),
            "mlpb1": np.ascontiguousarray(mlp_b1.reshape(F, 1)),
            "mlpW2": mlp_W2.astype(BF),
            "mlpb2r": np.ascontiguousarray(
                np.broadcast_to(mlp_b2, (128, NCLS)).astype(np.float32)),
            "invcntr": np.ascontiguousarray(
                np.broadcast_to(invcnt, (128, NGRAPH))),
        })
    return meta, in_maps


_CACHED = {}


def kernel_run(inputs, trace=False):
    meta, in_maps = _prepare(inputs)
    key = meta["TT"]
    if key not in _CACHED:
        _CACHED[key] = _build_program(meta)
    nc = _CACHED[key]
    res = run_bass_kernel_spmd(nc, in_maps, core_ids=list(range(NCORES)),
                               trace=trace)
    out = np.asarray(res.results[0]["out"], dtype=np.float32)
    return out, res.exec_time_ns


def kernel(**inputs):
    out, _ = kernel_run(inputs, trace=False)
    return out
